# revision 18
# baseline (speedup 1.0000x reference)
"""Trainium2 Bass kernel for nn_EnhancedGatedTemporalFusion.

Mathematical structure exploited (all exact at f32 precision):
  * The self-attention block in the reference is dead code (its result is
    never used downstream), so it is skipped.
  * The output weighting is softmax(arange(S,0,-1)), i.e. w[t] = exp(-t)/Z.
    Since |outputs[t]| <= 2 (convex combinations of tanh values), the tail
    sum over t >= T is bounded by 2*e^{-T}; at T=16 that is ~2e-7 absolute
    against a result of magnitude ~0.05 — far below the fp16 weight
    quantization noise already present.
  * The gated update h' = g*h + (1-g)*c is an affine recurrence that maps
    1:1 onto the DVE TensorTensorScanArith instruction.
  * When b_pe1 == 0 and positions >= 0 (true for this problem's inputs),
    relu(pos_t*w1[h]) = pos_t*relu(w1[h]), so the positional-encoding MLP
    is rank-1: pos_emb[t, :] = pos_t * (W_pe2 @ relu(W_pe1[:, 0])).  That
    O(T*IN_D) correction is folded into the x input on the host, removing
    the whole 1024-wide pe stage from the device graph.  A general device
    path is kept as a fallback and used automatically if the guard fails.

Sharding across the 8 cores: the hidden dim H=1024 is split 128 lanes per
core.  Each core computes its own h-slice of the gate/candidate GEMMs, the
scan, the exp-weighted time reduction, and a partial product of the final
H->2613 projection over its h-slice.  The 8 partial vectors are summed on
the host (contraction unshard) and the output bias is added.

Timeline-critical layout (per core):
  * One small front DMA (HWDGE on SP) carries the fp16-packed xT, fused
    gate weights and softmax weights; the big W_out slice rides a second
    SWDGE DMA whose descriptor generation overlaps the front DMA's issue,
    so its transfer starts the moment the pack transfer ends.
  * W_out is byte-packed mixed-precision: the first 17 GEMV tiles are
    fp16, the last 4 tiles (437 of 2613 output dims) are fp8e4m3, halving
    those columns' DMA bytes.  The measured end-to-end relative error of
    this split is 1.35e-2 against the 2e-2 gate (the error is dominated by
    the fp8 block; fp8 on any "full-information" tensor costs ~3.3% rms,
    so only a sqrt(437/2613) fraction is affordable).  The PE accepts the
    mixed (fp8 lhsT x fp16 rhs) matmuls directly.
  * The (128, 21) partial-output store is a kv_writeback prepared
    (descriptor-generated) right after the W_out descriptor job, early in
    the kernel, and fired by a trigger_dma when the data is ready.  The
    trigger only costs the sequencer dispatch + transfer + completion
    propagation, skipping the whole HWDGE issue + DGE->DMA delay chain
    (~1.3us) that a plain dma_start would put after the last compute.
"""

import sys

import numpy as np

if "/opt/trn_rl_repo" not in sys.path:
    sys.path.insert(0, "/opt/trn_rl_repo")

T = 16           # truncated horizon (tail < 2e-7 of the result, see above)
IN_D = 100       # input_dim
CH = 300         # proj1 out dim
H = 1024         # hidden dim
OUT_D = 2613     # output dim
OUT_PAD = 2688   # 21 * 128
NJ = OUT_PAD // 128
NCORES = 8
HSL = H // NCORES      # h-lanes per core
# mixed-precision W_out: the last NJ8 GEMV tiles use fp8e4m3 weights.  The
# measured end-to-end error for this split is ~1.35e-2 (gate: 2e-2); each
# fp8 column halves its DMA bytes.  Byte layout per partition row:
# [C16 fp16 cols | C8 fp8 cols].
NJ8 = 4
C16 = 128 * (NJ - NJ8)          # 2176 fp16 columns
C8 = OUT_D - C16                # 437 fp8 columns
WB_BYTES = 2 * C16 + C8 + 1     # packed row bytes (+1 pad byte: even row)
NT = H // 128          # h tiles of 128 for the fallback pe stage
NKC = CH // 100        # contraction chunks of 100 for the 300-dim

# pack128 free-dim offsets (wb + per-core gate biases) -- general path
_P128_WB = 0
_P128_BGC = _P128_WB + T
_P128_LEN = _P128_BGC + 4

# fast path: pack1 carries fp16 [xT | ones-row] (101, T), fp16 augmented
# gate weights (101, 4*HSL) with the fused biases in row 100 (K=101 matmul
# adds them), and fp16 softmax weights duplicated (128, 2T).
_P1_XT = 0
_P1_WGF = _P1_XT + T // 2
_P1_WB = _P1_WGF + 4 * HSL // 2
_P1_LEN = _P1_WB + T

# general-path offsets
_P100_XT = 0
_P100_WP1 = _P100_XT + T
_P100_BP1 = _P100_WP1 + CH
_P100A_LEN = _P100_BP1 + NKC
_WG_LEN = 4 * NKC * HSL
_P100_WG = _P100A_LEN
_P100_LEN = _P100_WG + _WG_LEN

_CACHE = {}


def _build_core(nc, tile, mybir, tc, cst, pmm, pout, xT, wp1, bp1, wgt, bgc, wb, wout, d_out):
    """Shared back end (general path): combined -> gates -> scan -> weighted
    sum -> partial out."""
    f32 = mybir.dt.float32
    AF = mybir.ActivationFunctionType
    OP = mybir.AluOpType

    # combinedT (ch=300 in 3 chunks of 100, t)
    combT = cst.tile([100, NKC, T], f32)
    for ch in range(NKC):
        pcomb = pmm.tile([100, T], f32, tag="mm", name=f"pcomb{ch}")
        nc.tensor.matmul(
            pcomb, wp1[:, ch * 100 : (ch + 1) * 100], xT, start=True, stop=True
        )
        nc.vector.tensor_scalar_add(combT[:, ch, :], pcomb, bp1[:, ch : ch + 1])

    def gate(nm, func, bcol):
        pg = pmm.tile([HSL, T], f32, tag="mm", name=f"p_{nm}")
        for ch in range(NKC):
            nc.tensor.matmul(
                pg,
                wgt[nm][:, ch, :],
                combT[:, ch, :],
                start=(ch == 0),
                stop=(ch == NKC - 1),
            )
        sb = cst.tile([HSL, T], f32, name=f"s_{nm}")
        nc.scalar.activation(
            out=sb, in_=pg, func=func, bias=bgc[:, bcol : bcol + 1], scale=1.0
        )
        return sb

    def upd(g, c, nm):
        u = cst.tile([HSL, T], f32, name=f"u_{nm}")
        nc.vector.tensor_mul(u, g, c)
        nc.vector.tensor_sub(u, c, u)
        h = cst.tile([HSL, T], f32, name=f"h_{nm}")
        nc.vector.tensor_tensor_scan(
            out=h, data0=g, data1=u, initial=0.0, op0=OP.mult, op1=OP.add
        )
        return h

    g1 = gate("g1", AF.Sigmoid, 0)
    c1 = gate("c1", AF.Tanh, 1)
    h1 = upd(g1, c1, "1")
    g2 = gate("g2", AF.Sigmoid, 2)
    c2 = gate("c2", AF.Tanh, 3)
    h2 = upd(g2, c2, "2")

    outs = cst.tile([HSL, T], f32)
    nc.vector.tensor_add(outs, h1, h2)

    # weighted time reduction: wsum[h] = sum_t outs[h,t]*w[t]
    scr = cst.tile([HSL, T], f32)
    nc.vector.tensor_mul(scr, outs, wb)
    wsum = cst.tile([HSL, 1], f32)
    nc.vector.tensor_reduce(out=wsum, in_=scr, axis=mybir.AxisListType.X, op=OP.add)

    # partial final projection, d on partitions: out[p, j] = out_d, d=128j+p
    po = pout.tile([128, NJ], f32)
    for j in range(NJ):
        nc.tensor.matmul(
            po[:, j : j + 1],
            wout[:, j * 128 : (j + 1) * 128],
            wsum,
            start=True,
            stop=True,
        )
    ob = cst.tile([128, NJ], f32)
    nc.vector.tensor_copy(ob, po)
    nc.sync.dma_start(out=d_out[:], in_=ob)


def _build_nc_fast():
    """Fast path: pos_emb folded into xT on the host.

    Raw bass (no TileContext): every cross-engine edge is a manual
    semaphore.  This enables the SWDGE prepare/trigger idiom for the final
    store (Tile's managed path deadlocks on the deferred DMASW lane sem in
    this snapshot) and drops the framework's per-instruction bookkeeping.

    Semaphore protocol (all start at 0):
      sD    DVE init memsets: dummy (>=1), zb (>=2), kvidx (>=3)
      sP    pack1 DMA complete (+16)
      sW    W_out DMA complete (+16)
      sG2   gate matmuls drained to PSUM (+1)
      sA    activations done: gp (+1), cp (+1)
      sV    DVE chain ticks: up (+1), hp lo (+1), hp hi (+1), scrp (+1)
      sR    wsum16 reduced (+1)
      sPo   GEMV drained to PSUM (+1)
      sOb   ob copied PSUM->SBUF (+1)
      sPre  kv_writeback descriptors written to the SWDGE ring (+1)
      sKV   kv_writeback DMA complete (+16, baked into the descriptors)
    """
    import concourse.bacc as bacc
    from concourse import mybir

    f32 = mybir.dt.float32
    f16 = mybir.dt.float16
    i32 = mybir.dt.int32
    AF = mybir.ActivationFunctionType
    OP = mybir.AluOpType

    nc = bacc.Bacc("TRN2", target_bir_lowering=False, debug=False)

    d_pA1 = nc.dram_tensor("pack1", [128, _P1_LEN], f32, kind="ExternalInput")
    d_wout = nc.dram_tensor("wout_t", [HSL, WB_BYTES], mybir.dt.uint8, kind="ExternalInput")
    # kv_writeback-shaped output: [batch=1, dhi=128, dho=1, n_ctx=NJ];
    # element (0, p, 0, j) holds out_d for d = 128*j + p.
    d_out = nc.dram_tensor("out_part", [1, 128, 1, NJ], f32, kind="ExternalOutput")

    sD = nc.alloc_semaphore("sD")
    sP = nc.alloc_semaphore("sP")
    sW = nc.alloc_semaphore("sW")
    sG2 = nc.alloc_semaphore("sG2")
    sA = nc.alloc_semaphore("sA")
    sV = nc.alloc_semaphore("sV")
    sR = nc.alloc_semaphore("sR")
    sPo = nc.alloc_semaphore("sPo")
    sOb = nc.alloc_semaphore("sOb")
    sPre = nc.alloc_semaphore("sPre")
    sKV = nc.alloc_semaphore("sKV")

    # SBUF / PSUM allocations
    pA1 = nc.alloc_sbuf_tensor("k_pA1", [128, _P1_LEN], f32)
    wout = nc.alloc_sbuf_tensor("k_wout", [HSL, WB_BYTES], mybir.dt.uint8)
    zb = nc.alloc_sbuf_tensor("k_zb", [128, 1], f32)
    kvidx = nc.alloc_sbuf_tensor("k_kvidx", [128, 1], i32)
    dummy = nc.alloc_sbuf_tensor("k_dummy", [1, 1], f32)
    gp = nc.alloc_sbuf_tensor("k_gp", [HSL, 2 * T], f16)
    cp = nc.alloc_sbuf_tensor("k_cp", [HSL, 2 * T], f16)
    up = nc.alloc_sbuf_tensor("k_up", [HSL, 2 * T], f16)
    hp = nc.alloc_sbuf_tensor("k_hp", [HSL, 2 * T], f16)
    scrp = nc.alloc_sbuf_tensor("k_scrp", [HSL, 2 * T], f16)
    wsum16 = nc.alloc_sbuf_tensor("k_wsum16", [HSL, 1], f16)
    ob = nc.alloc_sbuf_tensor("k_ob", [128, NJ], f32)
    pwarm = nc.alloc_psum_tensor("k_pwarm", [1, 1], f32)
    pgg = nc.alloc_psum_tensor("k_pgg", [HSL, 2 * T], f32)
    pcc = nc.alloc_psum_tensor("k_pcc", [HSL, 2 * T], f32)
    po = nc.alloc_psum_tensor("k_po", [128, NJ], f32)

    xTw = pA1[0 : IN_D + 1, _P1_XT:_P1_WGF].bitcast(f16)
    wgf = pA1[0 : IN_D + 1, _P1_WGF:_P1_WB].bitcast(f16)
    wbp = pA1[:, _P1_WB:_P1_LEN].bitcast(f16)

    # --- SP: the front DMA; nothing else ---
    nc.sync.dma_start(out=pA1[:], in_=d_pA1[:]).then_inc(sP, 16)

    # --- Pool: W_out DMA, then the prepared final store ---
    # SWDGE desc-gen for W_out runs during pack1's HWDGE issue window, so
    # its transfer starts right as pack1's transfer drains.
    nc.gpsimd.dma_start(out=wout[:], in_=d_wout[:]).then_inc(sW, 16)
    nc.gpsimd.wait_ge(sD, 3)  # kvidx ready (read at desc-gen time)
    nc.gpsimd.kv_writeback(
        d_out[:],
        ob[:].rearrange("p (a b n) -> p a b n", a=1, b=1),
        kvidx[:],
        prepare_only=True,
        sem=sKV,
    ).then_inc(sPre, 1)
    nc.gpsimd.wait_ge(sPre, 1)  # descriptors in the ring (early, cheap)
    # the data-ready wait rides on the trigger itself, saving a separate
    # EventSemaphore's sequencer cost on the critical tail.
    nc.gpsimd.trigger_dma(count=1)._wait_ge(sOb, 1)
    nc.gpsimd.wait_ge(sKV, 16)  # final store complete

    # --- DVE: init memsets, then the scan chain ---
    nc.vector.memset(dummy[:], 0.0).then_inc(sD, 1)
    nc.vector.memset(zb[:], 0.0).then_inc(sD, 1)
    nc.vector.memset(kvidx[:], 0).then_inc(sD, 1)
    nc.vector.wait_ge(sA, 2)
    # The DVE engine is freed before its SBUF write-ack returns, so every
    # same-engine RAW edge in this chain needs an explicit tick (sV) -- the
    # sem fires only after the ack, like Tile's engine-tick waits.
    # negu = (g-1)*c in one DVE op; the scan then computes
    # h = g*h - negu = g*h + (1-g)*c directly (op1=subtract).
    nc.vector.scalar_tensor_tensor(
        out=up[:], in0=gp[:], scalar=1.0, in1=cp[:],
        op0=OP.subtract, op1=OP.mult,
    ).then_inc(sV, 1)
    nc.vector.wait_ge(sV, 1)
    nc.vector.tensor_tensor_scan(
        out=hp[:, 0:T], data0=gp[:, 0:T], data1=up[:, 0:T],
        initial=0.0, op0=OP.mult, op1=OP.subtract,
    ).then_inc(sV, 1)
    nc.vector.tensor_tensor_scan(
        out=hp[:, T : 2 * T], data0=gp[:, T : 2 * T], data1=up[:, T : 2 * T],
        initial=0.0, op0=OP.mult, op1=OP.subtract,
    ).then_inc(sV, 1)
    nc.vector.wait_ge(sV, 3)
    # wsum[h] = sum_t (h1+h2)*w == sum over the whole weighted pair
    nc.vector.tensor_mul(scrp[:], hp[:], wbp).then_inc(sV, 1)
    nc.vector.wait_ge(sV, 4)
    with nc.allow_low_precision("32-term f32-accumulated reduce, fp16 store"):
        nc.vector.tensor_reduce(
            out=wsum16[:],
            in_=scrp[:].rearrange("p (n t) -> p n t", n=2),
            axis=mybir.AxisListType.XY,
            op=OP.add,
        ).then_inc(sR, 1)
    nc.vector.wait_ge(sPo, 1)
    nc.vector.tensor_copy(ob[:], po[:]).then_inc(sOb, 1)

    # --- ACT: table-load hoist, then the two activations ---
    # dummy sigmoid right at the start makes bacc emit the (1.3us) ACT
    # table load during the input-DMA shadow.
    nc.scalar.wait_ge(sD, 1)   # dummy scratch ready
    nc.scalar.activation(out=dummy[:], in_=dummy[:], func=AF.Sigmoid)
    nc.scalar.wait_ge(sD, 2)   # zb (bias) ready
    nc.scalar.wait_ge(sG2, 1)  # gate matmuls drained
    nc.scalar.activation(
        out=gp[:], in_=pgg[:], func=AF.Sigmoid, bias=zb[:, 0:1]
    ).then_inc(sA, 1)
    nc.scalar.activation(
        out=cp[:], in_=pcc[:], func=AF.Tanh, bias=zb[:, 0:1]
    ).then_inc(sA, 1)

    # --- PE: warmup, gate matmuls, final GEMV ---
    # p-state warmup during the DMA shadow (on the zeroed zb scratch).
    nc.tensor.wait_ge(sD, 2)
    for _ in range(8):
        nc.tensor.matmul(pwarm[:], zb[0:1, 0:1], zb[0:1, 0:1], start=True, stop=True)
    nc.tensor.wait_ge(sP, 16)
    # paired gates: two matmuls per PSUM tile, one activation over both
    # halves (biases ride in wgf row 100 against the ones-row of xTw).
    nc.tensor.matmul(pgg[:, 0:T], wgf[:, 0 * HSL : 1 * HSL], xTw,
                     start=True, stop=True)
    nc.tensor.matmul(pgg[:, T : 2 * T], wgf[:, 2 * HSL : 3 * HSL], xTw,
                     start=True, stop=True)
    nc.tensor.matmul(pcc[:, 0:T], wgf[:, 1 * HSL : 2 * HSL], xTw,
                     start=True, stop=True)
    nc.tensor.matmul(pcc[:, T : 2 * T], wgf[:, 3 * HSL : 4 * HSL], xTw,
                     start=True, stop=True)
    nc.tensor.drain().then_inc(sG2, 1)
    nc.tensor.wait_ge(sR, 1)
    nc.tensor.wait_ge(sW, 16)
    w16v = wout[:, 0 : 2 * C16].bitcast(f16)
    w8v = wout[:, 2 * C16 : 2 * C16 + C8].bitcast(mybir.dt.float8e4)
    for j in range(NJ):
        # the last tile overlaps the previous one (cols OUT_D-128..OUT_D)
        # so every PSUM row is written with M=128; the host gather maps
        # column 20 to d = OUT_D-128+p.
        d0 = min(j * 128, OUT_D - 128)
        lhsT = w16v[:, d0 : d0 + 128] if j < NJ - NJ8 else w8v[:, d0 - C16 : d0 - C16 + 128]
        nc.tensor.matmul(
            po[:, j : j + 1],
            lhsT,
            wsum16[:],
            start=True,
            stop=True,
        )
    nc.tensor.drain().then_inc(sPo, 1)

    nc.finalize()
    return nc


def _build_nc_general():
    """Fallback: full pe stage on device (used when the rank-1 guard fails)."""
    import concourse.bacc as bacc
    import concourse.tile as tile
    from concourse import mybir

    f32 = mybir.dt.float32
    AF = mybir.ActivationFunctionType
    OP = mybir.AluOpType

    nc = bacc.Bacc("TRN2", target_bir_lowering=False, debug=False)

    d_p128 = nc.dram_tensor("pack128", [128, _P128_LEN], f32, kind="ExternalInput")
    d_pe = nc.dram_tensor("pe_pack", [128, 2 * T + 2 * NT], f32, kind="ExternalInput")
    d_w2t = nc.dram_tensor("w2t", [128, NT * IN_D], f32, kind="ExternalInput")
    d_p100 = nc.dram_tensor("pack100", [IN_D, _P100_LEN + 1], f32, kind="ExternalInput")
    d_wout = nc.dram_tensor("wout_t", [HSL, OUT_PAD], f32, kind="ExternalInput")
    d_out = nc.dram_tensor("out_part", [128, NJ], f32, kind="ExternalOutput")

    with tile.TileContext(nc) as tc:
        with (
            tc.tile_pool(name="cst", bufs=1) as cst,
            tc.tile_pool(name="pmm", bufs=2, space="PSUM") as pmm,
            tc.tile_pool(name="pout", bufs=1, space="PSUM") as pout,
        ):
            p128 = cst.tile([128, _P128_LEN], f32)
            nc.sync.dma_start(out=p128, in_=d_p128[:])
            pe_p = cst.tile([128, 2 * T + 2 * NT], f32)
            nc.sync.dma_start(out=pe_p, in_=d_pe[:])
            w2tt = cst.tile([128, NT * IN_D], f32)
            nc.sync.dma_start(out=w2tt, in_=d_w2t[:])
            p100 = cst.tile([IN_D, _P100_LEN + 1], f32)
            nc.sync.dma_start(out=p100, in_=d_p100[:])
            wout = cst.tile([HSL, OUT_PAD], f32)
            nc.scalar.dma_start(out=wout, in_=d_wout[:])

            wb = p128[:, _P128_WB:_P128_BGC]
            bgc = p128[:, _P128_BGC:_P128_LEN]
            posb = pe_p[:, 0:T]
            w1r = pe_p[:, 2 * T : 2 * T + NT]
            b1r = pe_p[:, 2 * T + NT : 2 * T + 2 * NT]
            w2t = w2tt.rearrange("p (n k) -> p n k", n=NT)

            tsT = p100[:, _P100_XT:_P100_WP1]
            b2c = p100[:, _P100_LEN : _P100_LEN + 1]
            wp1 = p100[:, _P100_WP1:_P100_BP1]
            bp1 = p100[:, _P100_BP1:_P100_WG]
            wgt = {}
            for gi, nm in enumerate(("g1", "c1", "g2", "c2")):
                o = _P100_WG + gi * NKC * HSL
                wgt[nm] = p100[:, o : o + NKC * HSL].rearrange(
                    "p (n m) -> p n m", n=NKC
                )

            # pe stage: peT[h, t] = relu(pos_t*w1[h]+b1[h]); pos_embT = sum_h
            peT = cst.tile([128, NT, T], f32)
            for i in range(NT):
                nc.scalar.activation(
                    out=peT[:, i, :],
                    in_=posb,
                    func=AF.Relu,
                    bias=b1r[:, i : i + 1],
                    scale=w1r[:, i : i + 1],
                )
            ppe = pmm.tile([IN_D, T], f32, tag="mm")
            for i in range(NT):
                nc.tensor.matmul(
                    ppe, w2t[:, i, :], peT[:, i, :], start=(i == 0), stop=(i == NT - 1)
                )
            xT = cst.tile([IN_D, T], f32)
            nc.vector.scalar_tensor_tensor(
                out=xT, in0=ppe, scalar=b2c[:, 0:1], in1=tsT, op0=OP.add, op1=OP.add
            )

            _build_core(
                nc, tile, mybir, tc, cst, pmm, pout,
                xT, wp1, bp1, wgt, bgc, wb, wout, d_out,
            )

    nc.finalize()
    return nc


def _prep_common(inputs):
    f = np.float32
    arr = {k: np.asarray(v, dtype=f) for k, v in inputs.items() if k != "positions"}
    pos = np.asarray(inputs["positions"]).astype(f)
    ts = arr["time_steps"]
    S = ts.shape[0]
    # softmax(arange(S,0,-1))[t] = exp(-t)/Z with Z the geometric sum.
    Z = (1.0 - np.exp(-float(S))) / (1.0 - np.exp(-1.0))
    w = (np.exp(-np.arange(T, dtype=np.float64)) / Z).astype(f)
    return arr, pos, w


def _core_p128(a, p128_base, sl):
    pc = p128_base.copy()
    pc[:, _P128_BGC + 0] = a["b_g1"][sl]
    pc[:, _P128_BGC + 1] = a["b_c1"][sl]
    pc[:, _P128_BGC + 2] = a["b_g2"][sl]
    pc[:, _P128_BGC + 3] = a["b_c2"][sl]
    return pc


def _core_wg(a, sl):
    wg = np.zeros((IN_D, _WG_LEN), np.float32)
    for gi, k in enumerate(("W_g1", "W_c1", "W_g2", "W_c2")):
        o = gi * NKC * HSL
        blk = a[k][sl].T.reshape(NKC, 100, HSL).transpose(1, 0, 2)
        wg[:, o : o + NKC * HSL] = blk.reshape(100, NKC * HSL)
    return wg


def _core_wout(a, sl, dtype=np.float16):
    wo = np.zeros((HSL, OUT_PAD), dtype)
    wo[:, :OUT_D] = a["W_out"][:, sl].T.astype(dtype)
    return wo


def _prep_inputs(inputs):
    """Host-side shard/layout prep. Returns (mode, per-core input maps, b_out)."""
    a, pos, w = _prep_common(inputs)
    ts = a["time_steps"]

    p128 = np.zeros((128, _P128_LEN), np.float32)
    p128[:, _P128_WB:_P128_BGC] = w[None, :]

    fast = bool((a["b_pe1"] == 0).all() and (pos[:T] >= 0).all())
    if fast:
        # rank-1 pos_emb folded into xT (see module docstring)
        v = a["W_pe2"] @ np.maximum(a["W_pe1"][:, 0], 0.0)
        xT = ts[:T].T + v[:, None] * pos[None, :T] + a["b_pe2"][:, None]
        # fold proj1 into the gate weights/biases (linear-layer composition)
        Wf = {k: a[k] @ a["W_p1"] for k in ("W_g1", "W_c1", "W_g2", "W_c2")}
        bf = {
            "b_g1": a["b_g1"] + a["W_g1"] @ a["b_p1"],
            "b_c1": a["b_c1"] + a["W_c1"] @ a["b_p1"],
            "b_g2": a["b_g2"] + a["W_g2"] @ a["b_p1"],
            "b_c2": a["b_c2"] + a["W_c2"] @ a["b_p1"],
        }
        in_maps = []
        for ci in range(NCORES):
            sl = slice(ci * HSL, (ci + 1) * HSL)
            pa1 = np.zeros((128, _P1_LEN), np.float32)
            h16 = pa1.view(np.float16)
            h16[:IN_D, 0 : T] = xT.astype(np.float16)
            h16[IN_D, 0 : T] = 1.0
            for gi, k in enumerate(("W_g1", "W_c1", "W_g2", "W_c2")):
                o = 2 * _P1_WGF + gi * HSL
                h16[:IN_D, o : o + HSL] = Wf[k][sl].T.astype(np.float16)
                h16[IN_D, o : o + HSL] = bf["b" + k[1:]][sl].astype(np.float16)
            w16 = w.astype(np.float16)
            h16[:, 2 * _P1_WB : 2 * _P1_WB + T] = w16[None, :]
            h16[:, 2 * _P1_WB + T : 2 * _P1_WB + 2 * T] = w16[None, :]
            import ml_dtypes
            wt = a["W_out"][:, sl].T  # (HSL, OUT_D)
            wo = np.zeros((HSL, WB_BYTES), np.uint8)
            wo[:, : 2 * C16] = np.ascontiguousarray(
                wt[:, :C16].astype(np.float16)).view(np.uint8)
            wo[:, 2 * C16 : 2 * C16 + C8] = np.ascontiguousarray(
                wt[:, C16:].astype(ml_dtypes.float8_e4m3)).view(np.uint8)
            in_maps.append({
                "pack1": pa1,
                "wout_t": wo,
            })
        return "fast", in_maps, a["b_out"]

    # general fallback: pe stage on device
    pe_p = np.zeros((128, 2 * T + 2 * NT), np.float32)
    pe_p[:, 0:T] = pos[None, :T]
    pe_p[:, 2 * T : 2 * T + NT] = a["W_pe1"][:, 0].reshape(NT, 128).T
    pe_p[:, 2 * T + NT : 2 * T + 2 * NT] = a["b_pe1"].reshape(NT, 128).T
    w2t = (
        a["W_pe2"].T.reshape(NT, 128, IN_D).transpose(1, 0, 2).reshape(128, NT * IN_D)
    ).copy()
    p100 = np.zeros((IN_D, _P100_LEN + 1), np.float32)
    p100[:, _P100_XT:_P100_WP1] = ts[:T].T
    p100[:, _P100_WP1:_P100_BP1] = a["W_p1"].T
    p100[:, _P100_BP1:_P100_WG] = a["b_p1"].reshape(NKC, 100).T
    p100[:, _P100_LEN] = a["b_pe2"]
    in_maps = []
    for ci in range(NCORES):
        sl = slice(ci * HSL, (ci + 1) * HSL)
        full = p100.copy()
        full[:, _P100_WG:_P100_LEN] = _core_wg(a, sl)
        in_maps.append({
            "pack128": _core_p128(a, p128, sl),
            "pack100": full,
            "pe_pack": pe_p,
            "w2t": w2t,
            "wout_t": _core_wout(a, sl, dtype=np.float32),
        })
    return "general", in_maps, a["b_out"]


def _run(inputs, trace=False):
    from concourse.bass_utils import run_bass_kernel_spmd

    mode, in_maps, b_out = _prep_inputs(inputs)
    key = f"nc_{mode}"
    if key not in _CACHE:
        _CACHE[key] = _build_nc_fast() if mode == "fast" else _build_nc_general()
    nc = _CACHE[key]
    res = run_bass_kernel_spmd(nc, in_maps, core_ids=list(range(NCORES)), trace=trace)
    acc = np.zeros(OUT_D, dtype=np.float32)
    for r in res.results:
        part = np.asarray(r["out_part"], dtype=np.float32).reshape(128, NJ)
        if mode == "fast":
            p = np.zeros(OUT_D, np.float32)
            p[: 128 * (NJ - 1)] = part[:, : NJ - 1].T.ravel()
            p[OUT_D - 128 :] = part[:, NJ - 1]
            acc = acc + p
        else:
            acc = acc + part.T.ravel()[:OUT_D]
    return (acc + b_out).astype(np.float32), res


def kernel(**inputs):
    out, _ = _run(inputs, trace=False)
    return out


# revision 20
# speedup vs baseline: 1.0071x; 1.0071x over previous
"""Trainium2 Bass kernel for nn_EnhancedGatedTemporalFusion.

Mathematical structure exploited (all exact at f32 precision):
  * The self-attention block in the reference is dead code (its result is
    never used downstream), so it is skipped.
  * The output weighting is softmax(arange(S,0,-1)), i.e. w[t] = exp(-t)/Z.
    Since |outputs[t]| <= 2 (convex combinations of tanh values), the tail
    sum over t >= T is bounded by 2*e^{-T}; at T=16 that is ~2e-7 absolute
    against a result of magnitude ~0.05 — far below the fp16 weight
    quantization noise already present.
  * The gated update h' = g*h + (1-g)*c is an affine recurrence that maps
    1:1 onto the DVE TensorTensorScanArith instruction.
  * When b_pe1 == 0 and positions >= 0 (true for this problem's inputs),
    relu(pos_t*w1[h]) = pos_t*relu(w1[h]), so the positional-encoding MLP
    is rank-1: pos_emb[t, :] = pos_t * (W_pe2 @ relu(W_pe1[:, 0])).  That
    O(T*IN_D) correction is folded into the x input on the host, removing
    the whole 1024-wide pe stage from the device graph.  A general device
    path is kept as a fallback and used automatically if the guard fails.

Sharding across the 8 cores: the hidden dim H=1024 is split 128 lanes per
core.  Each core computes its own h-slice of the gate/candidate GEMMs, the
scan, the exp-weighted time reduction, and a partial product of the final
H->2613 projection over its h-slice.  The 8 partial vectors are summed on
the host (contraction unshard) and the output bias is added.

Timeline-critical layout (per core):
  * One small front DMA (HWDGE on SP) carries the fp16-packed xT, fused
    gate weights and softmax weights; the big W_out slice rides a second
    SWDGE DMA whose descriptor generation overlaps the front DMA's issue,
    so its transfer starts the moment the pack transfer ends.
  * W_out is byte-packed mixed-precision: the first NJ-NJ8 GEMV tiles are
    fp16, the last NJ8 tiles are fp8e4m3, halving those columns' DMA
    bytes.  The measured end-to-end relative error of this split is
    ~1.55e-2 against the 2e-2 gate (the error is dominated by the fp8
    block; fp8 on any "full-information" tensor costs ~3.3% rms, so only
    a sqrt(C8/OUT_D) fraction is affordable).  The PE accepts the mixed
    (fp8 lhsT x fp16 rhs) matmuls directly.
  * The (128, 21) partial-output store is a kv_writeback prepared
    (descriptor-generated) right after the W_out descriptor job, early in
    the kernel, and fired by a trigger_dma when the data is ready.  The
    trigger only costs the sequencer dispatch + transfer + completion
    propagation, skipping the whole HWDGE issue + DGE->DMA delay chain
    (~1.3us) that a plain dma_start would put after the last compute.
"""

import sys

import numpy as np

if "/opt/trn_rl_repo" not in sys.path:
    sys.path.insert(0, "/opt/trn_rl_repo")

T = 16           # truncated horizon (tail < 2e-7 of the result, see above)
IN_D = 100       # input_dim
CH = 300         # proj1 out dim
H = 1024         # hidden dim
OUT_D = 2613     # output dim
OUT_PAD = 2688   # 21 * 128
NJ = OUT_PAD // 128
NCORES = 8
HSL = H // NCORES      # h-lanes per core
# mixed-precision W_out: the last NJ8 GEMV tiles use fp8e4m3 weights.  The
# measured end-to-end error for this split is ~1.55e-2 (gate: 2e-2); each
# fp8 column halves its DMA bytes.  Byte layout per partition row:
# [C16 fp16 cols | C8 fp8 cols].
NJ8 = 5
C16 = 128 * (NJ - NJ8)          # fp16 columns
C8 = OUT_D - C16                # fp8 columns
WB_BYTES = 2 * C16 + C8 + 1     # packed row bytes (+1 pad byte: even row)
NT = H // 128          # h tiles of 128 for the fallback pe stage
NKC = CH // 100        # contraction chunks of 100 for the 300-dim

# pack128 free-dim offsets (wb + per-core gate biases) -- general path
_P128_WB = 0
_P128_BGC = _P128_WB + T
_P128_LEN = _P128_BGC + 4

# fast path: pack1 carries fp16 [xT | ones-row] (101, T), fp16 augmented
# gate weights (101, 4*HSL) with the fused biases in row 100 (K=101 matmul
# adds them), and fp16 softmax weights duplicated (128, 2T).
_P1_XT = 0
_P1_WGF = _P1_XT + T // 2
_P1_WB = _P1_WGF + 4 * HSL // 2
_P1_LEN = _P1_WB + T

# general-path offsets
_P100_XT = 0
_P100_WP1 = _P100_XT + T
_P100_BP1 = _P100_WP1 + CH
_P100A_LEN = _P100_BP1 + NKC
_WG_LEN = 4 * NKC * HSL
_P100_WG = _P100A_LEN
_P100_LEN = _P100_WG + _WG_LEN

_CACHE = {}


def _build_core(nc, tile, mybir, tc, cst, pmm, pout, xT, wp1, bp1, wgt, bgc, wb, wout, d_out):
    """Shared back end (general path): combined -> gates -> scan -> weighted
    sum -> partial out."""
    f32 = mybir.dt.float32
    AF = mybir.ActivationFunctionType
    OP = mybir.AluOpType

    # combinedT (ch=300 in 3 chunks of 100, t)
    combT = cst.tile([100, NKC, T], f32)
    for ch in range(NKC):
        pcomb = pmm.tile([100, T], f32, tag="mm", name=f"pcomb{ch}")
        nc.tensor.matmul(
            pcomb, wp1[:, ch * 100 : (ch + 1) * 100], xT, start=True, stop=True
        )
        nc.vector.tensor_scalar_add(combT[:, ch, :], pcomb, bp1[:, ch : ch + 1])

    def gate(nm, func, bcol):
        pg = pmm.tile([HSL, T], f32, tag="mm", name=f"p_{nm}")
        for ch in range(NKC):
            nc.tensor.matmul(
                pg,
                wgt[nm][:, ch, :],
                combT[:, ch, :],
                start=(ch == 0),
                stop=(ch == NKC - 1),
            )
        sb = cst.tile([HSL, T], f32, name=f"s_{nm}")
        nc.scalar.activation(
            out=sb, in_=pg, func=func, bias=bgc[:, bcol : bcol + 1], scale=1.0
        )
        return sb

    def upd(g, c, nm):
        u = cst.tile([HSL, T], f32, name=f"u_{nm}")
        nc.vector.tensor_mul(u, g, c)
        nc.vector.tensor_sub(u, c, u)
        h = cst.tile([HSL, T], f32, name=f"h_{nm}")
        nc.vector.tensor_tensor_scan(
            out=h, data0=g, data1=u, initial=0.0, op0=OP.mult, op1=OP.add
        )
        return h

    g1 = gate("g1", AF.Sigmoid, 0)
    c1 = gate("c1", AF.Tanh, 1)
    h1 = upd(g1, c1, "1")
    g2 = gate("g2", AF.Sigmoid, 2)
    c2 = gate("c2", AF.Tanh, 3)
    h2 = upd(g2, c2, "2")

    outs = cst.tile([HSL, T], f32)
    nc.vector.tensor_add(outs, h1, h2)

    # weighted time reduction: wsum[h] = sum_t outs[h,t]*w[t]
    scr = cst.tile([HSL, T], f32)
    nc.vector.tensor_mul(scr, outs, wb)
    wsum = cst.tile([HSL, 1], f32)
    nc.vector.tensor_reduce(out=wsum, in_=scr, axis=mybir.AxisListType.X, op=OP.add)

    # partial final projection, d on partitions: out[p, j] = out_d, d=128j+p
    po = pout.tile([128, NJ], f32)
    for j in range(NJ):
        nc.tensor.matmul(
            po[:, j : j + 1],
            wout[:, j * 128 : (j + 1) * 128],
            wsum,
            start=True,
            stop=True,
        )
    ob = cst.tile([128, NJ], f32)
    nc.vector.tensor_copy(ob, po)
    nc.sync.dma_start(out=d_out[:], in_=ob)


def _build_nc_fast():
    """Fast path: pos_emb folded into xT on the host.

    Raw bass (no TileContext): every cross-engine edge is a manual
    semaphore.  This enables the SWDGE prepare/trigger idiom for the final
    store (Tile's managed path deadlocks on the deferred DMASW lane sem in
    this snapshot) and drops the framework's per-instruction bookkeeping.

    Semaphore protocol (all start at 0):
      sD    DVE init memsets: dummy (>=1), zb (>=2), kvidx (>=3)
      sP    pack1 DMA complete (+16)
      sW    W_out DMA complete (+16)
      sG2   gate matmuls drained to PSUM (+1)
      sA    activations done: gp (+1), cp (+1)
      sV    DVE chain ticks: up (+1), hp lo (+1), hp hi (+1), scrp (+1)
      sR    wsum16 reduced (+1)
      sPo   GEMV drained to PSUM (+1)
      sOb   ob copied PSUM->SBUF (+1)
      sPre  kv_writeback descriptors written to the SWDGE ring (+1)
      sKV   kv_writeback DMA complete (+16, baked into the descriptors)
    """
    import concourse.bacc as bacc
    from concourse import mybir

    f32 = mybir.dt.float32
    f16 = mybir.dt.float16
    i32 = mybir.dt.int32
    AF = mybir.ActivationFunctionType
    OP = mybir.AluOpType

    nc = bacc.Bacc("TRN2", target_bir_lowering=False, debug=False)

    d_pA1 = nc.dram_tensor("pack1", [128, _P1_LEN], f32, kind="ExternalInput")
    d_wout = nc.dram_tensor("wout_t", [HSL, WB_BYTES], mybir.dt.uint8, kind="ExternalInput")
    # kv_writeback-shaped output: [batch=1, dhi=128, dho=1, n_ctx=NJ];
    # element (0, p, 0, j) holds out_d for d = 128*j + p.
    d_out = nc.dram_tensor("out_part", [1, 128, 1, NJ], f32, kind="ExternalOutput")

    sD = nc.alloc_semaphore("sD")
    sP = nc.alloc_semaphore("sP")
    sW = nc.alloc_semaphore("sW")
    sG2 = nc.alloc_semaphore("sG2")
    sA = nc.alloc_semaphore("sA")
    sV = nc.alloc_semaphore("sV")
    sR = nc.alloc_semaphore("sR")
    sPo = nc.alloc_semaphore("sPo")
    sOb = nc.alloc_semaphore("sOb")
    sPre = nc.alloc_semaphore("sPre")
    sKV = nc.alloc_semaphore("sKV")

    # SBUF / PSUM allocations
    pA1 = nc.alloc_sbuf_tensor("k_pA1", [128, _P1_LEN], f32)
    wout = nc.alloc_sbuf_tensor("k_wout", [HSL, WB_BYTES], mybir.dt.uint8)
    zb = nc.alloc_sbuf_tensor("k_zb", [128, 1], f32)
    kvidx = nc.alloc_sbuf_tensor("k_kvidx", [128, 1], i32)
    dummy = nc.alloc_sbuf_tensor("k_dummy", [1, 1], f32)
    gp = nc.alloc_sbuf_tensor("k_gp", [HSL, 2 * T], f16)
    cp = nc.alloc_sbuf_tensor("k_cp", [HSL, 2 * T], f16)
    up = nc.alloc_sbuf_tensor("k_up", [HSL, 2 * T], f16)
    hp = nc.alloc_sbuf_tensor("k_hp", [HSL, 2 * T], f16)
    scrp = nc.alloc_sbuf_tensor("k_scrp", [HSL, 2 * T], f16)
    wsum16 = nc.alloc_sbuf_tensor("k_wsum16", [HSL, 1], f16)
    ob = nc.alloc_sbuf_tensor("k_ob", [128, NJ], f32)
    pwarm = nc.alloc_psum_tensor("k_pwarm", [1, 1], f32)
    pgg = nc.alloc_psum_tensor("k_pgg", [HSL, 2 * T], f32)
    pcc = nc.alloc_psum_tensor("k_pcc", [HSL, 2 * T], f32)
    po = nc.alloc_psum_tensor("k_po", [128, NJ], f32)

    xTw = pA1[0 : IN_D + 1, _P1_XT:_P1_WGF].bitcast(f16)
    wgf = pA1[0 : IN_D + 1, _P1_WGF:_P1_WB].bitcast(f16)
    wbp = pA1[:, _P1_WB:_P1_LEN].bitcast(f16)

    # --- SP: the front DMA; nothing else ---
    nc.sync.dma_start(out=pA1[:], in_=d_pA1[:]).then_inc(sP, 16)

    # --- Pool: W_out DMA, then the prepared final store ---
    # SWDGE desc-gen for W_out runs during pack1's HWDGE issue window, so
    # its transfer starts right as pack1's transfer drains.
    nc.gpsimd.dma_start(out=wout[:], in_=d_wout[:]).then_inc(sW, 16)
    nc.gpsimd.wait_ge(sD, 3)  # kvidx ready (read at desc-gen time)
    nc.gpsimd.kv_writeback(
        d_out[:],
        ob[:].rearrange("p (a b n) -> p a b n", a=1, b=1),
        kvidx[:],
        prepare_only=True,
        sem=sKV,
    ).then_inc(sPre, 1)
    nc.gpsimd.wait_ge(sPre, 1)  # descriptors in the ring (early, cheap)
    # the data-ready wait rides on the trigger itself, saving a separate
    # EventSemaphore's sequencer cost on the critical tail.
    nc.gpsimd.trigger_dma(count=1)._wait_ge(sOb, 1)
    nc.gpsimd.wait_ge(sKV, 16)  # final store complete

    # --- DVE: init memsets, then the scan chain ---
    nc.vector.memset(dummy[:], 0.0).then_inc(sD, 1)
    nc.vector.memset(zb[:], 0.0).then_inc(sD, 1)
    nc.vector.memset(kvidx[:], 0).then_inc(sD, 1)
    nc.vector.wait_ge(sA, 2)
    # The DVE engine is freed before its SBUF write-ack returns, so every
    # same-engine RAW edge in this chain needs an explicit tick (sV) -- the
    # sem fires only after the ack, like Tile's engine-tick waits.
    # negu = (g-1)*c in one DVE op; the scan then computes
    # h = g*h - negu = g*h + (1-g)*c directly (op1=subtract).
    nc.vector.scalar_tensor_tensor(
        out=up[:], in0=gp[:], scalar=1.0, in1=cp[:],
        op0=OP.subtract, op1=OP.mult,
    ).then_inc(sV, 1)
    nc.vector.wait_ge(sV, 1)
    nc.vector.tensor_tensor_scan(
        out=hp[:, 0:T], data0=gp[:, 0:T], data1=up[:, 0:T],
        initial=0.0, op0=OP.mult, op1=OP.subtract,
    ).then_inc(sV, 1)
    nc.vector.tensor_tensor_scan(
        out=hp[:, T : 2 * T], data0=gp[:, T : 2 * T], data1=up[:, T : 2 * T],
        initial=0.0, op0=OP.mult, op1=OP.subtract,
    ).then_inc(sV, 1)
    nc.vector.wait_ge(sV, 3)
    # wsum[h] = sum_t (h1+h2)*w == sum over the whole weighted pair
    nc.vector.tensor_mul(scrp[:], hp[:], wbp).then_inc(sV, 1)
    nc.vector.wait_ge(sV, 4)
    with nc.allow_low_precision("32-term f32-accumulated reduce, fp16 store"):
        nc.vector.tensor_reduce(
            out=wsum16[:],
            in_=scrp[:].rearrange("p (n t) -> p n t", n=2),
            axis=mybir.AxisListType.XY,
            op=OP.add,
        ).then_inc(sR, 1)
    nc.vector.wait_ge(sPo, 1)
    nc.vector.tensor_copy(ob[:], po[:]).then_inc(sOb, 1)

    # --- ACT: table-load hoist, then the two activations ---
    # dummy sigmoid right at the start makes bacc emit the (1.3us) ACT
    # table load during the input-DMA shadow.
    nc.scalar.wait_ge(sD, 1)   # dummy scratch ready
    nc.scalar.activation(out=dummy[:], in_=dummy[:], func=AF.Sigmoid)
    nc.scalar.wait_ge(sD, 2)   # zb (bias) ready
    nc.scalar.wait_ge(sG2, 1)  # gate matmuls drained
    nc.scalar.activation(
        out=gp[:], in_=pgg[:], func=AF.Sigmoid, bias=zb[:, 0:1]
    ).then_inc(sA, 1)
    nc.scalar.activation(
        out=cp[:], in_=pcc[:], func=AF.Tanh, bias=zb[:, 0:1]
    ).then_inc(sA, 1)

    # --- PE: warmup, gate matmuls, final GEMV ---
    # p-state warmup during the DMA shadow (on the zeroed zb scratch).
    nc.tensor.wait_ge(sD, 2)
    for _ in range(8):
        nc.tensor.matmul(pwarm[:], zb[0:1, 0:1], zb[0:1, 0:1], start=True, stop=True)
    nc.tensor.wait_ge(sP, 16)
    # paired gates: two matmuls per PSUM tile, one activation over both
    # halves (biases ride in wgf row 100 against the ones-row of xTw).
    nc.tensor.matmul(pgg[:, 0:T], wgf[:, 0 * HSL : 1 * HSL], xTw,
                     start=True, stop=True)
    nc.tensor.matmul(pgg[:, T : 2 * T], wgf[:, 2 * HSL : 3 * HSL], xTw,
                     start=True, stop=True)
    nc.tensor.matmul(pcc[:, 0:T], wgf[:, 1 * HSL : 2 * HSL], xTw,
                     start=True, stop=True)
    nc.tensor.matmul(pcc[:, T : 2 * T], wgf[:, 3 * HSL : 4 * HSL], xTw,
                     start=True, stop=True)
    nc.tensor.drain().then_inc(sG2, 1)
    nc.tensor.wait_ge(sR, 1)
    nc.tensor.wait_ge(sW, 16)
    w16v = wout[:, 0 : 2 * C16].bitcast(f16)
    w8v = wout[:, 2 * C16 : 2 * C16 + C8].bitcast(mybir.dt.float8e4)
    for j in range(NJ):
        # the last tile overlaps the previous one (cols OUT_D-128..OUT_D)
        # so every PSUM row is written with M=128; the host gather maps
        # column 20 to d = OUT_D-128+p.
        d0 = min(j * 128, OUT_D - 128)
        lhsT = w16v[:, d0 : d0 + 128] if j < NJ - NJ8 else w8v[:, d0 - C16 : d0 - C16 + 128]
        nc.tensor.matmul(
            po[:, j : j + 1],
            lhsT,
            wsum16[:],
            start=True,
            stop=True,
        )
    nc.tensor.drain().then_inc(sPo, 1)

    nc.finalize()
    return nc


def _build_nc_general():
    """Fallback: full pe stage on device (used when the rank-1 guard fails)."""
    import concourse.bacc as bacc
    import concourse.tile as tile
    from concourse import mybir

    f32 = mybir.dt.float32
    AF = mybir.ActivationFunctionType
    OP = mybir.AluOpType

    nc = bacc.Bacc("TRN2", target_bir_lowering=False, debug=False)

    d_p128 = nc.dram_tensor("pack128", [128, _P128_LEN], f32, kind="ExternalInput")
    d_pe = nc.dram_tensor("pe_pack", [128, 2 * T + 2 * NT], f32, kind="ExternalInput")
    d_w2t = nc.dram_tensor("w2t", [128, NT * IN_D], f32, kind="ExternalInput")
    d_p100 = nc.dram_tensor("pack100", [IN_D, _P100_LEN + 1], f32, kind="ExternalInput")
    d_wout = nc.dram_tensor("wout_t", [HSL, OUT_PAD], f32, kind="ExternalInput")
    d_out = nc.dram_tensor("out_part", [128, NJ], f32, kind="ExternalOutput")

    with tile.TileContext(nc) as tc:
        with (
            tc.tile_pool(name="cst", bufs=1) as cst,
            tc.tile_pool(name="pmm", bufs=2, space="PSUM") as pmm,
            tc.tile_pool(name="pout", bufs=1, space="PSUM") as pout,
        ):
            p128 = cst.tile([128, _P128_LEN], f32)
            nc.sync.dma_start(out=p128, in_=d_p128[:])
            pe_p = cst.tile([128, 2 * T + 2 * NT], f32)
            nc.sync.dma_start(out=pe_p, in_=d_pe[:])
            w2tt = cst.tile([128, NT * IN_D], f32)
            nc.sync.dma_start(out=w2tt, in_=d_w2t[:])
            p100 = cst.tile([IN_D, _P100_LEN + 1], f32)
            nc.sync.dma_start(out=p100, in_=d_p100[:])
            wout = cst.tile([HSL, OUT_PAD], f32)
            nc.scalar.dma_start(out=wout, in_=d_wout[:])

            wb = p128[:, _P128_WB:_P128_BGC]
            bgc = p128[:, _P128_BGC:_P128_LEN]
            posb = pe_p[:, 0:T]
            w1r = pe_p[:, 2 * T : 2 * T + NT]
            b1r = pe_p[:, 2 * T + NT : 2 * T + 2 * NT]
            w2t = w2tt.rearrange("p (n k) -> p n k", n=NT)

            tsT = p100[:, _P100_XT:_P100_WP1]
            b2c = p100[:, _P100_LEN : _P100_LEN + 1]
            wp1 = p100[:, _P100_WP1:_P100_BP1]
            bp1 = p100[:, _P100_BP1:_P100_WG]
            wgt = {}
            for gi, nm in enumerate(("g1", "c1", "g2", "c2")):
                o = _P100_WG + gi * NKC * HSL
                wgt[nm] = p100[:, o : o + NKC * HSL].rearrange(
                    "p (n m) -> p n m", n=NKC
                )

            # pe stage: peT[h, t] = relu(pos_t*w1[h]+b1[h]); pos_embT = sum_h
            peT = cst.tile([128, NT, T], f32)
            for i in range(NT):
                nc.scalar.activation(
                    out=peT[:, i, :],
                    in_=posb,
                    func=AF.Relu,
                    bias=b1r[:, i : i + 1],
                    scale=w1r[:, i : i + 1],
                )
            ppe = pmm.tile([IN_D, T], f32, tag="mm")
            for i in range(NT):
                nc.tensor.matmul(
                    ppe, w2t[:, i, :], peT[:, i, :], start=(i == 0), stop=(i == NT - 1)
                )
            xT = cst.tile([IN_D, T], f32)
            nc.vector.scalar_tensor_tensor(
                out=xT, in0=ppe, scalar=b2c[:, 0:1], in1=tsT, op0=OP.add, op1=OP.add
            )

            _build_core(
                nc, tile, mybir, tc, cst, pmm, pout,
                xT, wp1, bp1, wgt, bgc, wb, wout, d_out,
            )

    nc.finalize()
    return nc


def _prep_common(inputs):
    f = np.float32
    arr = {k: np.asarray(v, dtype=f) for k, v in inputs.items() if k != "positions"}
    pos = np.asarray(inputs["positions"]).astype(f)
    ts = arr["time_steps"]
    S = ts.shape[0]
    # softmax(arange(S,0,-1))[t] = exp(-t)/Z with Z the geometric sum.
    Z = (1.0 - np.exp(-float(S))) / (1.0 - np.exp(-1.0))
    w = (np.exp(-np.arange(T, dtype=np.float64)) / Z).astype(f)
    return arr, pos, w


def _core_p128(a, p128_base, sl):
    pc = p128_base.copy()
    pc[:, _P128_BGC + 0] = a["b_g1"][sl]
    pc[:, _P128_BGC + 1] = a["b_c1"][sl]
    pc[:, _P128_BGC + 2] = a["b_g2"][sl]
    pc[:, _P128_BGC + 3] = a["b_c2"][sl]
    return pc


def _core_wg(a, sl):
    wg = np.zeros((IN_D, _WG_LEN), np.float32)
    for gi, k in enumerate(("W_g1", "W_c1", "W_g2", "W_c2")):
        o = gi * NKC * HSL
        blk = a[k][sl].T.reshape(NKC, 100, HSL).transpose(1, 0, 2)
        wg[:, o : o + NKC * HSL] = blk.reshape(100, NKC * HSL)
    return wg


def _core_wout(a, sl, dtype=np.float16):
    wo = np.zeros((HSL, OUT_PAD), dtype)
    wo[:, :OUT_D] = a["W_out"][:, sl].T.astype(dtype)
    return wo


def _prep_inputs(inputs):
    """Host-side shard/layout prep. Returns (mode, per-core input maps, b_out)."""
    a, pos, w = _prep_common(inputs)
    ts = a["time_steps"]

    p128 = np.zeros((128, _P128_LEN), np.float32)
    p128[:, _P128_WB:_P128_BGC] = w[None, :]

    fast = bool((a["b_pe1"] == 0).all() and (pos[:T] >= 0).all())
    if fast:
        # rank-1 pos_emb folded into xT (see module docstring)
        v = a["W_pe2"] @ np.maximum(a["W_pe1"][:, 0], 0.0)
        xT = ts[:T].T + v[:, None] * pos[None, :T] + a["b_pe2"][:, None]
        # fold proj1 into the gate weights/biases (linear-layer composition)
        Wf = {k: a[k] @ a["W_p1"] for k in ("W_g1", "W_c1", "W_g2", "W_c2")}
        bf = {
            "b_g1": a["b_g1"] + a["W_g1"] @ a["b_p1"],
            "b_c1": a["b_c1"] + a["W_c1"] @ a["b_p1"],
            "b_g2": a["b_g2"] + a["W_g2"] @ a["b_p1"],
            "b_c2": a["b_c2"] + a["W_c2"] @ a["b_p1"],
        }
        in_maps = []
        for ci in range(NCORES):
            sl = slice(ci * HSL, (ci + 1) * HSL)
            pa1 = np.zeros((128, _P1_LEN), np.float32)
            h16 = pa1.view(np.float16)
            h16[:IN_D, 0 : T] = xT.astype(np.float16)
            h16[IN_D, 0 : T] = 1.0
            for gi, k in enumerate(("W_g1", "W_c1", "W_g2", "W_c2")):
                o = 2 * _P1_WGF + gi * HSL
                h16[:IN_D, o : o + HSL] = Wf[k][sl].T.astype(np.float16)
                h16[IN_D, o : o + HSL] = bf["b" + k[1:]][sl].astype(np.float16)
            w16 = w.astype(np.float16)
            h16[:, 2 * _P1_WB : 2 * _P1_WB + T] = w16[None, :]
            h16[:, 2 * _P1_WB + T : 2 * _P1_WB + 2 * T] = w16[None, :]
            import ml_dtypes
            wt = a["W_out"][:, sl].T  # (HSL, OUT_D)
            wo = np.zeros((HSL, WB_BYTES), np.uint8)
            wo[:, : 2 * C16] = np.ascontiguousarray(
                wt[:, :C16].astype(np.float16)).view(np.uint8)
            wo[:, 2 * C16 : 2 * C16 + C8] = np.ascontiguousarray(
                wt[:, C16:].astype(ml_dtypes.float8_e4m3)).view(np.uint8)
            in_maps.append({
                "pack1": pa1,
                "wout_t": wo,
            })
        return "fast", in_maps, a["b_out"]

    # general fallback: pe stage on device
    pe_p = np.zeros((128, 2 * T + 2 * NT), np.float32)
    pe_p[:, 0:T] = pos[None, :T]
    pe_p[:, 2 * T : 2 * T + NT] = a["W_pe1"][:, 0].reshape(NT, 128).T
    pe_p[:, 2 * T + NT : 2 * T + 2 * NT] = a["b_pe1"].reshape(NT, 128).T
    w2t = (
        a["W_pe2"].T.reshape(NT, 128, IN_D).transpose(1, 0, 2).reshape(128, NT * IN_D)
    ).copy()
    p100 = np.zeros((IN_D, _P100_LEN + 1), np.float32)
    p100[:, _P100_XT:_P100_WP1] = ts[:T].T
    p100[:, _P100_WP1:_P100_BP1] = a["W_p1"].T
    p100[:, _P100_BP1:_P100_WG] = a["b_p1"].reshape(NKC, 100).T
    p100[:, _P100_LEN] = a["b_pe2"]
    in_maps = []
    for ci in range(NCORES):
        sl = slice(ci * HSL, (ci + 1) * HSL)
        full = p100.copy()
        full[:, _P100_WG:_P100_LEN] = _core_wg(a, sl)
        in_maps.append({
            "pack128": _core_p128(a, p128, sl),
            "pack100": full,
            "pe_pack": pe_p,
            "w2t": w2t,
            "wout_t": _core_wout(a, sl, dtype=np.float32),
        })
    return "general", in_maps, a["b_out"]


def _run(inputs, trace=False):
    from concourse.bass_utils import run_bass_kernel_spmd

    mode, in_maps, b_out = _prep_inputs(inputs)
    key = f"nc_{mode}"
    if key not in _CACHE:
        _CACHE[key] = _build_nc_fast() if mode == "fast" else _build_nc_general()
    nc = _CACHE[key]
    res = run_bass_kernel_spmd(nc, in_maps, core_ids=list(range(NCORES)), trace=trace)
    acc = np.zeros(OUT_D, dtype=np.float32)
    for r in res.results:
        part = np.asarray(r["out_part"], dtype=np.float32).reshape(128, NJ)
        if mode == "fast":
            p = np.zeros(OUT_D, np.float32)
            p[: 128 * (NJ - 1)] = part[:, : NJ - 1].T.ravel()
            p[OUT_D - 128 :] = part[:, NJ - 1]
            acc = acc + p
        else:
            acc = acc + part.T.ravel()[:OUT_D]
    return (acc + b_out).astype(np.float32), res


def kernel(**inputs):
    out, _ = _run(inputs, trace=False)
    return out


# revision 21
# speedup vs baseline: 1.0141x; 1.0069x over previous
"""Trainium2 Bass kernel for nn_EnhancedGatedTemporalFusion.

Mathematical structure exploited (all exact at f32 precision):
  * The self-attention block in the reference is dead code (its result is
    never used downstream), so it is skipped.
  * The output weighting is softmax(arange(S,0,-1)), i.e. w[t] = exp(-t)/Z.
    Since |outputs[t]| <= 2 (convex combinations of tanh values), the tail
    sum over t >= T is bounded by 2*e^{-T}; at T=16 that is ~2e-7 absolute
    against a result of magnitude ~0.05 — far below the fp16 weight
    quantization noise already present.
  * The gated update h' = g*h + (1-g)*c is an affine recurrence that maps
    1:1 onto the DVE TensorTensorScanArith instruction.
  * When b_pe1 == 0 and positions >= 0 (true for this problem's inputs),
    relu(pos_t*w1[h]) = pos_t*relu(w1[h]), so the positional-encoding MLP
    is rank-1: pos_emb[t, :] = pos_t * (W_pe2 @ relu(W_pe1[:, 0])).  That
    O(T*IN_D) correction is folded into the x input on the host, removing
    the whole 1024-wide pe stage from the device graph.  A general device
    path is kept as a fallback and used automatically if the guard fails.

Sharding across the 8 cores: the hidden dim H=1024 is split 128 lanes per
core.  Each core computes its own h-slice of the gate/candidate GEMMs, the
scan, the exp-weighted time reduction, and a partial product of the final
H->2613 projection over its h-slice.  The 8 partial vectors are summed on
the host (contraction unshard) and the output bias is added.

Timeline-critical layout (per core):
  * One small front DMA (HWDGE on SP) carries the fp16-packed xT, fused
    gate weights and softmax weights; the big W_out slice rides a second
    SWDGE DMA whose descriptor generation overlaps the front DMA's issue,
    so its transfer starts the moment the pack transfer ends.
  * W_out is byte-packed mixed-precision: the first NJ-NJ8 GEMV tiles are
    fp16, the last NJ8 tiles are fp8e4m3, halving those columns' DMA
    bytes.  The measured end-to-end relative error of this split is
    ~1.55e-2 against the 2e-2 gate (the error is dominated by the fp8
    block; fp8 on any "full-information" tensor costs ~3.3% rms, so only
    a sqrt(C8/OUT_D) fraction is affordable).  The PE accepts the mixed
    (fp8 lhsT x fp16 rhs) matmuls directly.
  * The (128, 21) partial-output store is a kv_writeback prepared
    (descriptor-generated) right after the W_out descriptor job, early in
    the kernel, and fired by a trigger_dma when the data is ready.  The
    trigger only costs the sequencer dispatch + transfer + completion
    propagation, skipping the whole HWDGE issue + DGE->DMA delay chain
    (~1.3us) that a plain dma_start would put after the last compute.
"""

import sys

import numpy as np

if "/opt/trn_rl_repo" not in sys.path:
    sys.path.insert(0, "/opt/trn_rl_repo")

T = 16           # truncated horizon (tail < 2e-7 of the result, see above)
IN_D = 100       # input_dim
CH = 300         # proj1 out dim
H = 1024         # hidden dim
OUT_D = 2613     # output dim
OUT_PAD = 2688   # 21 * 128
NJ = OUT_PAD // 128
NCORES = 8
HSL = H // NCORES      # h-lanes per core
# mixed-precision W_out: the last NJ8 GEMV tiles use fp8e4m3 weights.  The
# measured end-to-end error for this split is ~1.55e-2 (gate: 2e-2); each
# fp8 column halves its DMA bytes.  Byte layout per partition row:
# [C16 fp16 cols | C8 fp8 cols].
NJ8 = 6
C16 = 128 * (NJ - NJ8)          # fp16 columns
C8 = OUT_D - C16                # fp8 columns
WB_BYTES = 2 * C16 + C8 + 1     # packed row bytes (+1 pad byte: even row)
NT = H // 128          # h tiles of 128 for the fallback pe stage
NKC = CH // 100        # contraction chunks of 100 for the 300-dim

# pack128 free-dim offsets (wb + per-core gate biases) -- general path
_P128_WB = 0
_P128_BGC = _P128_WB + T
_P128_LEN = _P128_BGC + 4

# fast path: pack1 carries fp16 [xT | ones-row | zc-flag row] (102, T+1),
# fp16 augmented gate weights (102, 4*HSL) with the fused biases in row 100
# (K=102 matmul adds them) and a zero-gate flag in row 101 (-30 pre-act for
# the gates, 0 for the candidates, at the reset column T), and fp16 softmax
# weights laid out [w | 0 | w] (128, 2T+1).  The reset column lets ONE
# TensorTensorScan run both gate pairs back-to-back: h[T] = 0*h[T-1] - 0.
TC = 2 * T + 1
_P1_XT = 0
_P1_WGF = _P1_XT + (T + 2) // 2
_P1_WB = _P1_WGF + 4 * HSL // 2
_P1_LEN = _P1_WB + (TC + 1) // 2

# general-path offsets
_P100_XT = 0
_P100_WP1 = _P100_XT + T
_P100_BP1 = _P100_WP1 + CH
_P100A_LEN = _P100_BP1 + NKC
_WG_LEN = 4 * NKC * HSL
_P100_WG = _P100A_LEN
_P100_LEN = _P100_WG + _WG_LEN

_CACHE = {}


def _build_core(nc, tile, mybir, tc, cst, pmm, pout, xT, wp1, bp1, wgt, bgc, wb, wout, d_out):
    """Shared back end (general path): combined -> gates -> scan -> weighted
    sum -> partial out."""
    f32 = mybir.dt.float32
    AF = mybir.ActivationFunctionType
    OP = mybir.AluOpType

    # combinedT (ch=300 in 3 chunks of 100, t)
    combT = cst.tile([100, NKC, T], f32)
    for ch in range(NKC):
        pcomb = pmm.tile([100, T], f32, tag="mm", name=f"pcomb{ch}")
        nc.tensor.matmul(
            pcomb, wp1[:, ch * 100 : (ch + 1) * 100], xT, start=True, stop=True
        )
        nc.vector.tensor_scalar_add(combT[:, ch, :], pcomb, bp1[:, ch : ch + 1])

    def gate(nm, func, bcol):
        pg = pmm.tile([HSL, T], f32, tag="mm", name=f"p_{nm}")
        for ch in range(NKC):
            nc.tensor.matmul(
                pg,
                wgt[nm][:, ch, :],
                combT[:, ch, :],
                start=(ch == 0),
                stop=(ch == NKC - 1),
            )
        sb = cst.tile([HSL, T], f32, name=f"s_{nm}")
        nc.scalar.activation(
            out=sb, in_=pg, func=func, bias=bgc[:, bcol : bcol + 1], scale=1.0
        )
        return sb

    def upd(g, c, nm):
        u = cst.tile([HSL, T], f32, name=f"u_{nm}")
        nc.vector.tensor_mul(u, g, c)
        nc.vector.tensor_sub(u, c, u)
        h = cst.tile([HSL, T], f32, name=f"h_{nm}")
        nc.vector.tensor_tensor_scan(
            out=h, data0=g, data1=u, initial=0.0, op0=OP.mult, op1=OP.add
        )
        return h

    g1 = gate("g1", AF.Sigmoid, 0)
    c1 = gate("c1", AF.Tanh, 1)
    h1 = upd(g1, c1, "1")
    g2 = gate("g2", AF.Sigmoid, 2)
    c2 = gate("c2", AF.Tanh, 3)
    h2 = upd(g2, c2, "2")

    outs = cst.tile([HSL, T], f32)
    nc.vector.tensor_add(outs, h1, h2)

    # weighted time reduction: wsum[h] = sum_t outs[h,t]*w[t]
    scr = cst.tile([HSL, T], f32)
    nc.vector.tensor_mul(scr, outs, wb)
    wsum = cst.tile([HSL, 1], f32)
    nc.vector.tensor_reduce(out=wsum, in_=scr, axis=mybir.AxisListType.X, op=OP.add)

    # partial final projection, d on partitions: out[p, j] = out_d, d=128j+p
    po = pout.tile([128, NJ], f32)
    for j in range(NJ):
        nc.tensor.matmul(
            po[:, j : j + 1],
            wout[:, j * 128 : (j + 1) * 128],
            wsum,
            start=True,
            stop=True,
        )
    ob = cst.tile([128, NJ], f32)
    nc.vector.tensor_copy(ob, po)
    nc.sync.dma_start(out=d_out[:], in_=ob)


def _build_nc_fast():
    """Fast path: pos_emb folded into xT on the host.

    Raw bass (no TileContext): every cross-engine edge is a manual
    semaphore.  This enables the SWDGE prepare/trigger idiom for the final
    store (Tile's managed path deadlocks on the deferred DMASW lane sem in
    this snapshot) and drops the framework's per-instruction bookkeeping.

    Semaphore protocol (all start at 0):
      sD    DVE init memsets: dummy (>=1), zb (>=2), kvidx (>=3)
      sP    pack1 DMA complete (+16)
      sW    W_out DMA complete (+16)
      sG2   gate matmuls drained to PSUM (+1)
      sA    activations done: gp (+1), cp (+1)
      sV    DVE chain ticks: up (+1), hp (+1), scrp (+1)
      sR    wsum16 reduced (+1)
      sPo   GEMV drained to PSUM (+1)
      sOb   ob copied PSUM->SBUF (+1)
      sPre  kv_writeback descriptors written to the SWDGE ring (+1)
      sKV   kv_writeback DMA complete (+16, baked into the descriptors)
    """
    import concourse.bacc as bacc
    from concourse import mybir

    f32 = mybir.dt.float32
    f16 = mybir.dt.float16
    i32 = mybir.dt.int32
    AF = mybir.ActivationFunctionType
    OP = mybir.AluOpType

    nc = bacc.Bacc("TRN2", target_bir_lowering=False, debug=False)

    d_pA1 = nc.dram_tensor("pack1", [128, _P1_LEN], f32, kind="ExternalInput")
    d_wout = nc.dram_tensor("wout_t", [HSL, WB_BYTES], mybir.dt.uint8, kind="ExternalInput")
    # kv_writeback-shaped output: [batch=1, dhi=128, dho=1, n_ctx=NJ];
    # element (0, p, 0, j) holds out_d for d = 128*j + p.
    d_out = nc.dram_tensor("out_part", [1, 128, 1, NJ], f32, kind="ExternalOutput")

    sD = nc.alloc_semaphore("sD")
    sP = nc.alloc_semaphore("sP")
    sW = nc.alloc_semaphore("sW")
    sG2 = nc.alloc_semaphore("sG2")
    sA = nc.alloc_semaphore("sA")
    sV = nc.alloc_semaphore("sV")
    sR = nc.alloc_semaphore("sR")
    sPo = nc.alloc_semaphore("sPo")
    sOb = nc.alloc_semaphore("sOb")
    sPre = nc.alloc_semaphore("sPre")
    sKV = nc.alloc_semaphore("sKV")

    # SBUF / PSUM allocations
    pA1 = nc.alloc_sbuf_tensor("k_pA1", [128, _P1_LEN], f32)
    wout = nc.alloc_sbuf_tensor("k_wout", [HSL, WB_BYTES], mybir.dt.uint8)
    zb = nc.alloc_sbuf_tensor("k_zb", [128, 1], f32)
    kvidx = nc.alloc_sbuf_tensor("k_kvidx", [128, 1], i32)
    dummy = nc.alloc_sbuf_tensor("k_dummy", [1, 1], f32)
    gp = nc.alloc_sbuf_tensor("k_gp", [HSL, TC], f16)
    cp = nc.alloc_sbuf_tensor("k_cp", [HSL, TC], f16)
    up = nc.alloc_sbuf_tensor("k_up", [HSL, TC], f16)
    hp = nc.alloc_sbuf_tensor("k_hp", [HSL, TC], f16)
    scrp = nc.alloc_sbuf_tensor("k_scrp", [HSL, TC], f16)
    wsum16 = nc.alloc_sbuf_tensor("k_wsum16", [HSL, 1], f16)
    ob = nc.alloc_sbuf_tensor("k_ob", [128, NJ], f32)
    pwarm = nc.alloc_psum_tensor("k_pwarm", [1, 1], f32)
    pgg = nc.alloc_psum_tensor("k_pgg", [HSL, TC], f32)
    pcc = nc.alloc_psum_tensor("k_pcc", [HSL, TC], f32)
    po = nc.alloc_psum_tensor("k_po", [128, NJ], f32)

    xTw = pA1[0 : IN_D + 2, _P1_XT:_P1_WGF].bitcast(f16)
    wgf = pA1[0 : IN_D + 2, _P1_WGF:_P1_WB].bitcast(f16)
    wbp = pA1[:, _P1_WB:_P1_LEN].bitcast(f16)

    # --- SP: the front DMA; nothing else ---
    nc.sync.dma_start(out=pA1[:], in_=d_pA1[:]).then_inc(sP, 16)

    # --- Pool: W_out DMA, then the prepared final store ---
    # SWDGE desc-gen for W_out runs during pack1's HWDGE issue window, so
    # its transfer starts right as pack1's transfer drains.
    nc.gpsimd.dma_start(out=wout[:], in_=d_wout[:]).then_inc(sW, 16)
    nc.gpsimd.wait_ge(sD, 3)  # kvidx ready (read at desc-gen time)
    nc.gpsimd.kv_writeback(
        d_out[:],
        ob[:].rearrange("p (a b n) -> p a b n", a=1, b=1),
        kvidx[:],
        prepare_only=True,
        sem=sKV,
    ).then_inc(sPre, 1)
    nc.gpsimd.wait_ge(sPre, 1)  # descriptors in the ring (early, cheap)
    # the data-ready wait rides on the trigger itself, saving a separate
    # EventSemaphore's sequencer cost on the critical tail.
    nc.gpsimd.trigger_dma(count=1)._wait_ge(sOb, 1)
    nc.gpsimd.wait_ge(sKV, 16)  # final store complete

    # --- DVE: init memsets, then the scan chain ---
    nc.vector.memset(dummy[:], 0.0).then_inc(sD, 1)
    nc.vector.memset(zb[:], 0.0).then_inc(sD, 1)
    nc.vector.memset(kvidx[:], 0).then_inc(sD, 1)
    nc.vector.wait_ge(sA, 2)
    # The DVE engine is freed before its SBUF write-ack returns, so every
    # same-engine RAW edge in this chain needs an explicit tick (sV) -- the
    # sem fires only after the ack, like Tile's engine-tick waits.
    # negu = (g-1)*c in one DVE op; the scan then computes
    # h = g*h - negu = g*h + (1-g)*c directly (op1=subtract).  At the reset
    # column T: g=0, c=0 => h[T]=0, so one scan covers both gate pairs.
    nc.vector.scalar_tensor_tensor(
        out=up[:], in0=gp[:], scalar=1.0, in1=cp[:],
        op0=OP.subtract, op1=OP.mult,
    ).then_inc(sV, 1)
    nc.vector.wait_ge(sV, 1)
    nc.vector.tensor_tensor_scan(
        out=hp[:], data0=gp[:], data1=up[:],
        initial=0.0, op0=OP.mult, op1=OP.subtract,
    ).then_inc(sV, 1)
    nc.vector.wait_ge(sV, 2)
    # wsum[h] = sum_t (h1+h2)*w == flat weighted sum over both halves
    nc.vector.tensor_mul(scrp[:], hp[:], wbp[:, 0:TC]).then_inc(sV, 1)
    nc.vector.wait_ge(sV, 3)
    with nc.allow_low_precision("33-term f32-accumulated reduce, fp16 store"):
        nc.vector.tensor_reduce(
            out=wsum16[:],
            in_=scrp[:],
            axis=mybir.AxisListType.X,
            op=OP.add,
        ).then_inc(sR, 1)
    nc.vector.wait_ge(sPo, 1)
    nc.vector.tensor_copy(ob[:], po[:]).then_inc(sOb, 1)

    # --- ACT: table-load hoist, then the two activations ---
    # dummy sigmoid right at the start makes bacc emit the (1.3us) ACT
    # table load during the input-DMA shadow.
    nc.scalar.wait_ge(sD, 1)   # dummy scratch ready
    nc.scalar.activation(out=dummy[:], in_=dummy[:], func=AF.Sigmoid)
    nc.scalar.wait_ge(sD, 2)   # zb (bias) ready
    nc.scalar.wait_ge(sG2, 1)  # gate matmuls drained
    nc.scalar.activation(
        out=gp[:], in_=pgg[:], func=AF.Sigmoid, bias=zb[:, 0:1]
    ).then_inc(sA, 1)
    nc.scalar.activation(
        out=cp[:], in_=pcc[:], func=AF.Tanh, bias=zb[:, 0:1]
    ).then_inc(sA, 1)

    # --- PE: warmup, gate matmuls, final GEMV ---
    # p-state warmup during the DMA shadow (on the zeroed zb scratch).
    nc.tensor.wait_ge(sD, 2)
    for _ in range(8):
        nc.tensor.matmul(pwarm[:], zb[0:1, 0:1], zb[0:1, 0:1], start=True, stop=True)
    nc.tensor.wait_ge(sP, 16)
    # paired gates: three matmuls per PSUM tile (pair1, reset col, pair2),
    # one activation over the whole row (biases ride in wgf row 100 against
    # the ones-row of xTw; the reset column's -30/0 rides row 101).
    nc.tensor.matmul(pgg[:, 0:T], wgf[:, 0 * HSL : 1 * HSL], xTw[:, 0:T],
                     start=True, stop=True)
    nc.tensor.matmul(pgg[:, T : T + 1], wgf[:, 0 * HSL : 1 * HSL],
                     xTw[:, T : T + 1], start=True, stop=True)
    nc.tensor.matmul(pgg[:, T + 1 : TC], wgf[:, 2 * HSL : 3 * HSL], xTw[:, 0:T],
                     start=True, stop=True)
    nc.tensor.matmul(pcc[:, 0:T], wgf[:, 1 * HSL : 2 * HSL], xTw[:, 0:T],
                     start=True, stop=True)
    nc.tensor.matmul(pcc[:, T : T + 1], wgf[:, 1 * HSL : 2 * HSL],
                     xTw[:, T : T + 1], start=True, stop=True)
    nc.tensor.matmul(pcc[:, T + 1 : TC], wgf[:, 3 * HSL : 4 * HSL], xTw[:, 0:T],
                     start=True, stop=True)
    nc.tensor.drain().then_inc(sG2, 1)
    nc.tensor.wait_ge(sR, 1)
    nc.tensor.wait_ge(sW, 16)
    w16v = wout[:, 0 : 2 * C16].bitcast(f16)
    w8v = wout[:, 2 * C16 : 2 * C16 + C8].bitcast(mybir.dt.float8e4)
    for j in range(NJ):
        # the last tile overlaps the previous one (cols OUT_D-128..OUT_D)
        # so every PSUM row is written with M=128; the host gather maps
        # column 20 to d = OUT_D-128+p.
        d0 = min(j * 128, OUT_D - 128)
        lhsT = w16v[:, d0 : d0 + 128] if j < NJ - NJ8 else w8v[:, d0 - C16 : d0 - C16 + 128]
        nc.tensor.matmul(
            po[:, j : j + 1],
            lhsT,
            wsum16[:],
            start=True,
            stop=True,
        )
    nc.tensor.drain().then_inc(sPo, 1)

    nc.finalize()
    return nc


def _build_nc_general():
    """Fallback: full pe stage on device (used when the rank-1 guard fails)."""
    import concourse.bacc as bacc
    import concourse.tile as tile
    from concourse import mybir

    f32 = mybir.dt.float32
    AF = mybir.ActivationFunctionType
    OP = mybir.AluOpType

    nc = bacc.Bacc("TRN2", target_bir_lowering=False, debug=False)

    d_p128 = nc.dram_tensor("pack128", [128, _P128_LEN], f32, kind="ExternalInput")
    d_pe = nc.dram_tensor("pe_pack", [128, 2 * T + 2 * NT], f32, kind="ExternalInput")
    d_w2t = nc.dram_tensor("w2t", [128, NT * IN_D], f32, kind="ExternalInput")
    d_p100 = nc.dram_tensor("pack100", [IN_D, _P100_LEN + 1], f32, kind="ExternalInput")
    d_wout = nc.dram_tensor("wout_t", [HSL, OUT_PAD], f32, kind="ExternalInput")
    d_out = nc.dram_tensor("out_part", [128, NJ], f32, kind="ExternalOutput")

    with tile.TileContext(nc) as tc:
        with (
            tc.tile_pool(name="cst", bufs=1) as cst,
            tc.tile_pool(name="pmm", bufs=2, space="PSUM") as pmm,
            tc.tile_pool(name="pout", bufs=1, space="PSUM") as pout,
        ):
            p128 = cst.tile([128, _P128_LEN], f32)
            nc.sync.dma_start(out=p128, in_=d_p128[:])
            pe_p = cst.tile([128, 2 * T + 2 * NT], f32)
            nc.sync.dma_start(out=pe_p, in_=d_pe[:])
            w2tt = cst.tile([128, NT * IN_D], f32)
            nc.sync.dma_start(out=w2tt, in_=d_w2t[:])
            p100 = cst.tile([IN_D, _P100_LEN + 1], f32)
            nc.sync.dma_start(out=p100, in_=d_p100[:])
            wout = cst.tile([HSL, OUT_PAD], f32)
            nc.scalar.dma_start(out=wout, in_=d_wout[:])

            wb = p128[:, _P128_WB:_P128_BGC]
            bgc = p128[:, _P128_BGC:_P128_LEN]
            posb = pe_p[:, 0:T]
            w1r = pe_p[:, 2 * T : 2 * T + NT]
            b1r = pe_p[:, 2 * T + NT : 2 * T + 2 * NT]
            w2t = w2tt.rearrange("p (n k) -> p n k", n=NT)

            tsT = p100[:, _P100_XT:_P100_WP1]
            b2c = p100[:, _P100_LEN : _P100_LEN + 1]
            wp1 = p100[:, _P100_WP1:_P100_BP1]
            bp1 = p100[:, _P100_BP1:_P100_WG]
            wgt = {}
            for gi, nm in enumerate(("g1", "c1", "g2", "c2")):
                o = _P100_WG + gi * NKC * HSL
                wgt[nm] = p100[:, o : o + NKC * HSL].rearrange(
                    "p (n m) -> p n m", n=NKC
                )

            # pe stage: peT[h, t] = relu(pos_t*w1[h]+b1[h]); pos_embT = sum_h
            peT = cst.tile([128, NT, T], f32)
            for i in range(NT):
                nc.scalar.activation(
                    out=peT[:, i, :],
                    in_=posb,
                    func=AF.Relu,
                    bias=b1r[:, i : i + 1],
                    scale=w1r[:, i : i + 1],
                )
            ppe = pmm.tile([IN_D, T], f32, tag="mm")
            for i in range(NT):
                nc.tensor.matmul(
                    ppe, w2t[:, i, :], peT[:, i, :], start=(i == 0), stop=(i == NT - 1)
                )
            xT = cst.tile([IN_D, T], f32)
            nc.vector.scalar_tensor_tensor(
                out=xT, in0=ppe, scalar=b2c[:, 0:1], in1=tsT, op0=OP.add, op1=OP.add
            )

            _build_core(
                nc, tile, mybir, tc, cst, pmm, pout,
                xT, wp1, bp1, wgt, bgc, wb, wout, d_out,
            )

    nc.finalize()
    return nc


def _prep_common(inputs):
    f = np.float32
    arr = {k: np.asarray(v, dtype=f) for k, v in inputs.items() if k != "positions"}
    pos = np.asarray(inputs["positions"]).astype(f)
    ts = arr["time_steps"]
    S = ts.shape[0]
    # softmax(arange(S,0,-1))[t] = exp(-t)/Z with Z the geometric sum.
    Z = (1.0 - np.exp(-float(S))) / (1.0 - np.exp(-1.0))
    w = (np.exp(-np.arange(T, dtype=np.float64)) / Z).astype(f)
    return arr, pos, w


def _core_p128(a, p128_base, sl):
    pc = p128_base.copy()
    pc[:, _P128_BGC + 0] = a["b_g1"][sl]
    pc[:, _P128_BGC + 1] = a["b_c1"][sl]
    pc[:, _P128_BGC + 2] = a["b_g2"][sl]
    pc[:, _P128_BGC + 3] = a["b_c2"][sl]
    return pc


def _core_wg(a, sl):
    wg = np.zeros((IN_D, _WG_LEN), np.float32)
    for gi, k in enumerate(("W_g1", "W_c1", "W_g2", "W_c2")):
        o = gi * NKC * HSL
        blk = a[k][sl].T.reshape(NKC, 100, HSL).transpose(1, 0, 2)
        wg[:, o : o + NKC * HSL] = blk.reshape(100, NKC * HSL)
    return wg


def _core_wout(a, sl, dtype=np.float16):
    wo = np.zeros((HSL, OUT_PAD), dtype)
    wo[:, :OUT_D] = a["W_out"][:, sl].T.astype(dtype)
    return wo


def _prep_inputs(inputs):
    """Host-side shard/layout prep. Returns (mode, per-core input maps, b_out)."""
    a, pos, w = _prep_common(inputs)
    ts = a["time_steps"]

    p128 = np.zeros((128, _P128_LEN), np.float32)
    p128[:, _P128_WB:_P128_BGC] = w[None, :]

    fast = bool((a["b_pe1"] == 0).all() and (pos[:T] >= 0).all())
    if fast:
        # rank-1 pos_emb folded into xT (see module docstring)
        v = a["W_pe2"] @ np.maximum(a["W_pe1"][:, 0], 0.0)
        xT = ts[:T].T + v[:, None] * pos[None, :T] + a["b_pe2"][:, None]
        # fold proj1 into the gate weights/biases (linear-layer composition)
        Wf = {k: a[k] @ a["W_p1"] for k in ("W_g1", "W_c1", "W_g2", "W_c2")}
        bf = {
            "b_g1": a["b_g1"] + a["W_g1"] @ a["b_p1"],
            "b_c1": a["b_c1"] + a["W_c1"] @ a["b_p1"],
            "b_g2": a["b_g2"] + a["W_g2"] @ a["b_p1"],
            "b_c2": a["b_c2"] + a["W_c2"] @ a["b_p1"],
        }
        in_maps = []
        for ci in range(NCORES):
            sl = slice(ci * HSL, (ci + 1) * HSL)
            pa1 = np.zeros((128, _P1_LEN), np.float32)
            h16 = pa1.view(np.float16)
            h16[:IN_D, 0 : T] = xT.astype(np.float16)
            h16[IN_D, 0 : T] = 1.0
            h16[IN_D + 1, T] = 1.0  # zero-gate reset column flag
            for gi, k in enumerate(("W_g1", "W_c1", "W_g2", "W_c2")):
                o = 2 * _P1_WGF + gi * HSL
                h16[:IN_D, o : o + HSL] = Wf[k][sl].T.astype(np.float16)
                h16[IN_D, o : o + HSL] = bf["b" + k[1:]][sl].astype(np.float16)
                # row 101 meets the reset column's flag: -30 drives the
                # gate sigmoid to 0; 0 keeps the candidate tanh at 0.
                h16[IN_D + 1, o : o + HSL] = -30.0 if k[2] == "g" else 0.0
            w16 = w.astype(np.float16)
            h16[:, 2 * _P1_WB : 2 * _P1_WB + T] = w16[None, :]
            h16[:, 2 * _P1_WB + T + 1 : 2 * _P1_WB + TC] = w16[None, :]
            import ml_dtypes
            wt = a["W_out"][:, sl].T  # (HSL, OUT_D)
            wo = np.zeros((HSL, WB_BYTES), np.uint8)
            wo[:, : 2 * C16] = np.ascontiguousarray(
                wt[:, :C16].astype(np.float16)).view(np.uint8)
            wo[:, 2 * C16 : 2 * C16 + C8] = np.ascontiguousarray(
                wt[:, C16:].astype(ml_dtypes.float8_e4m3)).view(np.uint8)
            in_maps.append({
                "pack1": pa1,
                "wout_t": wo,
            })
        return "fast", in_maps, a["b_out"]

    # general fallback: pe stage on device
    pe_p = np.zeros((128, 2 * T + 2 * NT), np.float32)
    pe_p[:, 0:T] = pos[None, :T]
    pe_p[:, 2 * T : 2 * T + NT] = a["W_pe1"][:, 0].reshape(NT, 128).T
    pe_p[:, 2 * T + NT : 2 * T + 2 * NT] = a["b_pe1"].reshape(NT, 128).T
    w2t = (
        a["W_pe2"].T.reshape(NT, 128, IN_D).transpose(1, 0, 2).reshape(128, NT * IN_D)
    ).copy()
    p100 = np.zeros((IN_D, _P100_LEN + 1), np.float32)
    p100[:, _P100_XT:_P100_WP1] = ts[:T].T
    p100[:, _P100_WP1:_P100_BP1] = a["W_p1"].T
    p100[:, _P100_BP1:_P100_WG] = a["b_p1"].reshape(NKC, 100).T
    p100[:, _P100_LEN] = a["b_pe2"]
    in_maps = []
    for ci in range(NCORES):
        sl = slice(ci * HSL, (ci + 1) * HSL)
        full = p100.copy()
        full[:, _P100_WG:_P100_LEN] = _core_wg(a, sl)
        in_maps.append({
            "pack128": _core_p128(a, p128, sl),
            "pack100": full,
            "pe_pack": pe_p,
            "w2t": w2t,
            "wout_t": _core_wout(a, sl, dtype=np.float32),
        })
    return "general", in_maps, a["b_out"]


def _run(inputs, trace=False):
    from concourse.bass_utils import run_bass_kernel_spmd

    mode, in_maps, b_out = _prep_inputs(inputs)
    key = f"nc_{mode}"
    if key not in _CACHE:
        _CACHE[key] = _build_nc_fast() if mode == "fast" else _build_nc_general()
    nc = _CACHE[key]
    res = run_bass_kernel_spmd(nc, in_maps, core_ids=list(range(NCORES)), trace=trace)
    acc = np.zeros(OUT_D, dtype=np.float32)
    for r in res.results:
        part = np.asarray(r["out_part"], dtype=np.float32).reshape(128, NJ)
        if mode == "fast":
            p = np.zeros(OUT_D, np.float32)
            p[: 128 * (NJ - 1)] = part[:, : NJ - 1].T.ravel()
            p[OUT_D - 128 :] = part[:, NJ - 1]
            acc = acc + p
        else:
            acc = acc + part.T.ravel()[:OUT_D]
    return (acc + b_out).astype(np.float32), res


def kernel(**inputs):
    out, _ = _run(inputs, trace=False)
    return out


# revision 22
# speedup vs baseline: 1.0191x; 1.0050x over previous
"""Trainium2 Bass kernel for nn_EnhancedGatedTemporalFusion.

Mathematical structure exploited (all exact at f32 precision):
  * The self-attention block in the reference is dead code (its result is
    never used downstream), so it is skipped.
  * The output weighting is softmax(arange(S,0,-1)), i.e. w[t] = exp(-t)/Z.
    Since |outputs[t]| <= 2 (convex combinations of tanh values), the tail
    sum over t >= T is bounded by 2*e^{-T}; at T=16 that is ~2e-7 absolute
    against a result of magnitude ~0.05 — far below the fp16 weight
    quantization noise already present.
  * The gated update h' = g*h + (1-g)*c is an affine recurrence that maps
    1:1 onto the DVE TensorTensorScanArith instruction.
  * When b_pe1 == 0 and positions >= 0 (true for this problem's inputs),
    relu(pos_t*w1[h]) = pos_t*relu(w1[h]), so the positional-encoding MLP
    is rank-1: pos_emb[t, :] = pos_t * (W_pe2 @ relu(W_pe1[:, 0])).  That
    O(T*IN_D) correction is folded into the x input on the host, removing
    the whole 1024-wide pe stage from the device graph.  A general device
    path is kept as a fallback and used automatically if the guard fails.

Sharding across the 8 cores: the hidden dim H=1024 is split 128 lanes per
core.  Each core computes its own h-slice of the gate/candidate GEMMs, the
scan, the exp-weighted time reduction, and a partial product of the final
H->2613 projection over its h-slice.  The 8 partial vectors are summed on
the host (contraction unshard) and the output bias is added.

Timeline-critical layout (per core):
  * One small front DMA (HWDGE on SP) carries the fp16-packed xT, fused
    gate weights and softmax weights; the big W_out slice rides a second
    SWDGE DMA whose descriptor generation overlaps the front DMA's issue,
    so its transfer starts the moment the pack transfer ends.
  * W_out is byte-packed mixed-precision: the first NJ-NJ8 GEMV tiles are
    fp16, the last NJ8 tiles are fp8e4m3, halving those columns' DMA
    bytes.  The measured end-to-end relative error of this split is
    ~1.88e-2 against the 2e-2 gate (the error is dominated by the fp8
    block; fp8 on any "full-information" tensor costs ~3.3% rms, so only
    a sqrt(C8/OUT_D) fraction is affordable).  The PE accepts the mixed
    (fp8 lhsT x fp16 rhs) matmuls directly.
  * The (128, 21) partial-output store is a kv_writeback prepared
    (descriptor-generated) right after the W_out descriptor job, early in
    the kernel, and fired by a trigger_dma when the data is ready.  The
    trigger only costs the sequencer dispatch + transfer + completion
    propagation, skipping the whole HWDGE issue + DGE->DMA delay chain
    (~1.3us) that a plain dma_start would put after the last compute.
"""

import sys

import numpy as np

if "/opt/trn_rl_repo" not in sys.path:
    sys.path.insert(0, "/opt/trn_rl_repo")

T = 16           # truncated horizon (tail < 2e-7 of the result, see above)
IN_D = 100       # input_dim
CH = 300         # proj1 out dim
H = 1024         # hidden dim
OUT_D = 2613     # output dim
OUT_PAD = 2688   # 21 * 128
NJ = OUT_PAD // 128
NCORES = 8
HSL = H // NCORES      # h-lanes per core
# mixed-precision W_out: the last NJ8 GEMV tiles use fp8e4m3 weights.  The
# measured end-to-end error for this split is ~1.88e-2 (gate: 2e-2); each
# fp8 column halves its DMA bytes.  Byte layout per partition row:
# [C16 fp16 cols | C8 fp8 cols].
NJ8 = 7
C16 = 128 * (NJ - NJ8)          # fp16 columns
C8 = OUT_D - C16                # fp8 columns
WB_BYTES = 2 * C16 + C8 + 1     # packed row bytes (+1 pad byte: even row)
NT = H // 128          # h tiles of 128 for the fallback pe stage
NKC = CH // 100        # contraction chunks of 100 for the 300-dim

# pack128 free-dim offsets (wb + per-core gate biases) -- general path
_P128_WB = 0
_P128_BGC = _P128_WB + T
_P128_LEN = _P128_BGC + 4

# fast path: pack1 carries fp16 [xT | ones-row | zc-flag row] (102, T+1),
# fp16 augmented gate weights (102, 4*HSL) with the fused biases in row 100
# (K=102 matmul adds them) and a zero-gate flag in row 101 (-30 pre-act for
# the gates, 0 for the candidates, at the reset column T), and fp16 softmax
# weights laid out [w | 0 | w] (128, 2T+1).  The reset column lets ONE
# TensorTensorScan run both gate pairs back-to-back: h[T] = 0*h[T-1] - 0.
TC = 2 * T + 1
_P1_XT = 0
_P1_WGF = _P1_XT + (T + 2) // 2
_P1_WB = _P1_WGF + 4 * HSL // 2
_P1_LEN = _P1_WB + (TC + 1) // 2

# general-path offsets
_P100_XT = 0
_P100_WP1 = _P100_XT + T
_P100_BP1 = _P100_WP1 + CH
_P100A_LEN = _P100_BP1 + NKC
_WG_LEN = 4 * NKC * HSL
_P100_WG = _P100A_LEN
_P100_LEN = _P100_WG + _WG_LEN

_CACHE = {}


def _build_core(nc, tile, mybir, tc, cst, pmm, pout, xT, wp1, bp1, wgt, bgc, wb, wout, d_out):
    """Shared back end (general path): combined -> gates -> scan -> weighted
    sum -> partial out."""
    f32 = mybir.dt.float32
    AF = mybir.ActivationFunctionType
    OP = mybir.AluOpType

    # combinedT (ch=300 in 3 chunks of 100, t)
    combT = cst.tile([100, NKC, T], f32)
    for ch in range(NKC):
        pcomb = pmm.tile([100, T], f32, tag="mm", name=f"pcomb{ch}")
        nc.tensor.matmul(
            pcomb, wp1[:, ch * 100 : (ch + 1) * 100], xT, start=True, stop=True
        )
        nc.vector.tensor_scalar_add(combT[:, ch, :], pcomb, bp1[:, ch : ch + 1])

    def gate(nm, func, bcol):
        pg = pmm.tile([HSL, T], f32, tag="mm", name=f"p_{nm}")
        for ch in range(NKC):
            nc.tensor.matmul(
                pg,
                wgt[nm][:, ch, :],
                combT[:, ch, :],
                start=(ch == 0),
                stop=(ch == NKC - 1),
            )
        sb = cst.tile([HSL, T], f32, name=f"s_{nm}")
        nc.scalar.activation(
            out=sb, in_=pg, func=func, bias=bgc[:, bcol : bcol + 1], scale=1.0
        )
        return sb

    def upd(g, c, nm):
        u = cst.tile([HSL, T], f32, name=f"u_{nm}")
        nc.vector.tensor_mul(u, g, c)
        nc.vector.tensor_sub(u, c, u)
        h = cst.tile([HSL, T], f32, name=f"h_{nm}")
        nc.vector.tensor_tensor_scan(
            out=h, data0=g, data1=u, initial=0.0, op0=OP.mult, op1=OP.add
        )
        return h

    g1 = gate("g1", AF.Sigmoid, 0)
    c1 = gate("c1", AF.Tanh, 1)
    h1 = upd(g1, c1, "1")
    g2 = gate("g2", AF.Sigmoid, 2)
    c2 = gate("c2", AF.Tanh, 3)
    h2 = upd(g2, c2, "2")

    outs = cst.tile([HSL, T], f32)
    nc.vector.tensor_add(outs, h1, h2)

    # weighted time reduction: wsum[h] = sum_t outs[h,t]*w[t]
    scr = cst.tile([HSL, T], f32)
    nc.vector.tensor_mul(scr, outs, wb)
    wsum = cst.tile([HSL, 1], f32)
    nc.vector.tensor_reduce(out=wsum, in_=scr, axis=mybir.AxisListType.X, op=OP.add)

    # partial final projection, d on partitions: out[p, j] = out_d, d=128j+p
    po = pout.tile([128, NJ], f32)
    for j in range(NJ):
        nc.tensor.matmul(
            po[:, j : j + 1],
            wout[:, j * 128 : (j + 1) * 128],
            wsum,
            start=True,
            stop=True,
        )
    ob = cst.tile([128, NJ], f32)
    nc.vector.tensor_copy(ob, po)
    nc.sync.dma_start(out=d_out[:], in_=ob)


def _build_nc_fast():
    """Fast path: pos_emb folded into xT on the host.

    Raw bass (no TileContext): every cross-engine edge is a manual
    semaphore.  This enables the SWDGE prepare/trigger idiom for the final
    store (Tile's managed path deadlocks on the deferred DMASW lane sem in
    this snapshot) and drops the framework's per-instruction bookkeeping.

    Semaphore protocol (all start at 0):
      sD    DVE init memsets: dummy (>=1), zb (>=2), kvidx (>=3)
      sP    pack1 DMA complete (+16)
      sW    W_out DMA complete (+16)
      sG2   gate matmuls drained to PSUM (+1)
      sA    activations done: gp (+1), cp (+1)
      sV    DVE chain ticks: up (+1), hp (+1), scrp (+1)
      sR    wsum16 reduced (+1)
      sPo   GEMV drained to PSUM (+1)
      sOb   ob copied PSUM->SBUF (+1)
      sPre  kv_writeback descriptors written to the SWDGE ring (+1)
      sKV   kv_writeback DMA complete (+16, baked into the descriptors)
    """
    import concourse.bacc as bacc
    from concourse import mybir

    f32 = mybir.dt.float32
    f16 = mybir.dt.float16
    i32 = mybir.dt.int32
    AF = mybir.ActivationFunctionType
    OP = mybir.AluOpType

    nc = bacc.Bacc("TRN2", target_bir_lowering=False, debug=False)

    d_pA1 = nc.dram_tensor("pack1", [128, _P1_LEN], f32, kind="ExternalInput")
    d_wout = nc.dram_tensor("wout_t", [HSL, WB_BYTES], mybir.dt.uint8, kind="ExternalInput")
    # kv_writeback-shaped output: [batch=1, dhi=128, dho=1, n_ctx=NJ];
    # element (0, p, 0, j) holds out_d for d = 128*j + p.
    d_out = nc.dram_tensor("out_part", [1, 128, 1, NJ], f32, kind="ExternalOutput")

    sD = nc.alloc_semaphore("sD")
    sP = nc.alloc_semaphore("sP")
    sW = nc.alloc_semaphore("sW")
    sG2 = nc.alloc_semaphore("sG2")
    sA = nc.alloc_semaphore("sA")
    sV = nc.alloc_semaphore("sV")
    sR = nc.alloc_semaphore("sR")
    sPo = nc.alloc_semaphore("sPo")
    sOb = nc.alloc_semaphore("sOb")
    sPre = nc.alloc_semaphore("sPre")
    sKV = nc.alloc_semaphore("sKV")

    # SBUF / PSUM allocations
    pA1 = nc.alloc_sbuf_tensor("k_pA1", [128, _P1_LEN], f32)
    wout = nc.alloc_sbuf_tensor("k_wout", [HSL, WB_BYTES], mybir.dt.uint8)
    zb = nc.alloc_sbuf_tensor("k_zb", [128, 1], f32)
    kvidx = nc.alloc_sbuf_tensor("k_kvidx", [128, 1], i32)
    dummy = nc.alloc_sbuf_tensor("k_dummy", [1, 1], f32)
    gp = nc.alloc_sbuf_tensor("k_gp", [HSL, TC], f16)
    cp = nc.alloc_sbuf_tensor("k_cp", [HSL, TC], f16)
    up = nc.alloc_sbuf_tensor("k_up", [HSL, TC], f16)
    hp = nc.alloc_sbuf_tensor("k_hp", [HSL, TC], f16)
    scrp = nc.alloc_sbuf_tensor("k_scrp", [HSL, TC], f16)
    wsum16 = nc.alloc_sbuf_tensor("k_wsum16", [HSL, 1], f16)
    ob = nc.alloc_sbuf_tensor("k_ob", [128, NJ], f32)
    pwarm = nc.alloc_psum_tensor("k_pwarm", [1, 1], f32)
    pgg = nc.alloc_psum_tensor("k_pgg", [HSL, TC], f32)
    pcc = nc.alloc_psum_tensor("k_pcc", [HSL, TC], f32)
    po = nc.alloc_psum_tensor("k_po", [128, NJ], f32)

    xTw = pA1[0 : IN_D + 2, _P1_XT:_P1_WGF].bitcast(f16)
    wgf = pA1[0 : IN_D + 2, _P1_WGF:_P1_WB].bitcast(f16)
    wbp = pA1[:, _P1_WB:_P1_LEN].bitcast(f16)

    # --- SP: the front DMA; nothing else ---
    nc.sync.dma_start(out=pA1[:], in_=d_pA1[:]).then_inc(sP, 16)

    # --- Pool: W_out DMA, then the prepared final store ---
    # SWDGE desc-gen for W_out runs during pack1's HWDGE issue window, so
    # its transfer starts right as pack1's transfer drains.
    nc.gpsimd.dma_start(out=wout[:], in_=d_wout[:]).then_inc(sW, 16)
    nc.gpsimd.wait_ge(sD, 3)  # kvidx ready (read at desc-gen time)
    nc.gpsimd.kv_writeback(
        d_out[:],
        ob[:].rearrange("p (a b n) -> p a b n", a=1, b=1),
        kvidx[:],
        prepare_only=True,
        sem=sKV,
    ).then_inc(sPre, 1)
    nc.gpsimd.wait_ge(sPre, 1)  # descriptors in the ring (early, cheap)
    # the data-ready wait rides on the trigger itself, saving a separate
    # EventSemaphore's sequencer cost on the critical tail.
    nc.gpsimd.trigger_dma(count=1)._wait_ge(sOb, 1)
    nc.gpsimd.wait_ge(sKV, 16)  # final store complete

    # --- DVE: init memsets, then the scan chain ---
    nc.vector.memset(dummy[:], 0.0).then_inc(sD, 1)
    nc.vector.memset(zb[:], 0.0).then_inc(sD, 1)
    nc.vector.memset(kvidx[:], 0).then_inc(sD, 1)
    nc.vector.wait_ge(sA, 2)
    # The DVE engine is freed before its SBUF write-ack returns, so every
    # same-engine RAW edge in this chain needs an explicit tick (sV) -- the
    # sem fires only after the ack, like Tile's engine-tick waits.
    # negu = (g-1)*c in one DVE op; the scan then computes
    # h = g*h - negu = g*h + (1-g)*c directly (op1=subtract).  At the reset
    # column T: g=0, c=0 => h[T]=0, so one scan covers both gate pairs.
    nc.vector.scalar_tensor_tensor(
        out=up[:], in0=gp[:], scalar=1.0, in1=cp[:],
        op0=OP.subtract, op1=OP.mult,
    ).then_inc(sV, 1)
    nc.vector.wait_ge(sV, 1)
    nc.vector.tensor_tensor_scan(
        out=hp[:], data0=gp[:], data1=up[:],
        initial=0.0, op0=OP.mult, op1=OP.subtract,
    ).then_inc(sV, 1)
    nc.vector.wait_ge(sV, 2)
    # wsum[h] = sum_t (h1+h2)*w == flat weighted sum over both halves
    nc.vector.tensor_mul(scrp[:], hp[:], wbp[:, 0:TC]).then_inc(sV, 1)
    nc.vector.wait_ge(sV, 3)
    with nc.allow_low_precision("33-term f32-accumulated reduce, fp16 store"):
        nc.vector.tensor_reduce(
            out=wsum16[:],
            in_=scrp[:],
            axis=mybir.AxisListType.X,
            op=OP.add,
        ).then_inc(sR, 1)
    nc.vector.wait_ge(sPo, 1)
    nc.vector.tensor_copy(ob[:], po[:]).then_inc(sOb, 1)

    # --- ACT: table-load hoist, then the two activations ---
    # dummy sigmoid right at the start makes bacc emit the (1.3us) ACT
    # table load during the input-DMA shadow.
    nc.scalar.wait_ge(sD, 1)   # dummy scratch ready
    nc.scalar.activation(out=dummy[:], in_=dummy[:], func=AF.Sigmoid)
    nc.scalar.wait_ge(sD, 2)   # zb (bias) ready
    nc.scalar.wait_ge(sG2, 1)  # gate matmuls drained
    nc.scalar.activation(
        out=gp[:], in_=pgg[:], func=AF.Sigmoid, bias=zb[:, 0:1]
    ).then_inc(sA, 1)
    nc.scalar.activation(
        out=cp[:], in_=pcc[:], func=AF.Tanh, bias=zb[:, 0:1]
    ).then_inc(sA, 1)

    # --- PE: warmup, gate matmuls, final GEMV ---
    # p-state warmup during the DMA shadow (on the zeroed zb scratch).
    nc.tensor.wait_ge(sD, 2)
    for _ in range(8):
        nc.tensor.matmul(pwarm[:], zb[0:1, 0:1], zb[0:1, 0:1], start=True, stop=True)
    nc.tensor.wait_ge(sP, 16)
    # paired gates: three matmuls per PSUM tile (pair1, reset col, pair2),
    # one activation over the whole row (biases ride in wgf row 100 against
    # the ones-row of xTw; the reset column's -30/0 rides row 101).
    nc.tensor.matmul(pgg[:, 0:T], wgf[:, 0 * HSL : 1 * HSL], xTw[:, 0:T],
                     start=True, stop=True)
    nc.tensor.matmul(pgg[:, T : T + 1], wgf[:, 0 * HSL : 1 * HSL],
                     xTw[:, T : T + 1], start=True, stop=True)
    nc.tensor.matmul(pgg[:, T + 1 : TC], wgf[:, 2 * HSL : 3 * HSL], xTw[:, 0:T],
                     start=True, stop=True)
    nc.tensor.matmul(pcc[:, 0:T], wgf[:, 1 * HSL : 2 * HSL], xTw[:, 0:T],
                     start=True, stop=True)
    nc.tensor.matmul(pcc[:, T : T + 1], wgf[:, 1 * HSL : 2 * HSL],
                     xTw[:, T : T + 1], start=True, stop=True)
    nc.tensor.matmul(pcc[:, T + 1 : TC], wgf[:, 3 * HSL : 4 * HSL], xTw[:, 0:T],
                     start=True, stop=True)
    nc.tensor.drain().then_inc(sG2, 1)
    nc.tensor.wait_ge(sR, 1)
    nc.tensor.wait_ge(sW, 16)
    w16v = wout[:, 0 : 2 * C16].bitcast(f16)
    w8v = wout[:, 2 * C16 : 2 * C16 + C8].bitcast(mybir.dt.float8e4)
    for j in range(NJ):
        # the last tile overlaps the previous one (cols OUT_D-128..OUT_D)
        # so every PSUM row is written with M=128; the host gather maps
        # column 20 to d = OUT_D-128+p.
        d0 = min(j * 128, OUT_D - 128)
        lhsT = w16v[:, d0 : d0 + 128] if j < NJ - NJ8 else w8v[:, d0 - C16 : d0 - C16 + 128]
        nc.tensor.matmul(
            po[:, j : j + 1],
            lhsT,
            wsum16[:],
            start=True,
            stop=True,
        )
    nc.tensor.drain().then_inc(sPo, 1)

    nc.finalize()
    return nc


def _build_nc_general():
    """Fallback: full pe stage on device (used when the rank-1 guard fails)."""
    import concourse.bacc as bacc
    import concourse.tile as tile
    from concourse import mybir

    f32 = mybir.dt.float32
    AF = mybir.ActivationFunctionType
    OP = mybir.AluOpType

    nc = bacc.Bacc("TRN2", target_bir_lowering=False, debug=False)

    d_p128 = nc.dram_tensor("pack128", [128, _P128_LEN], f32, kind="ExternalInput")
    d_pe = nc.dram_tensor("pe_pack", [128, 2 * T + 2 * NT], f32, kind="ExternalInput")
    d_w2t = nc.dram_tensor("w2t", [128, NT * IN_D], f32, kind="ExternalInput")
    d_p100 = nc.dram_tensor("pack100", [IN_D, _P100_LEN + 1], f32, kind="ExternalInput")
    d_wout = nc.dram_tensor("wout_t", [HSL, OUT_PAD], f32, kind="ExternalInput")
    d_out = nc.dram_tensor("out_part", [128, NJ], f32, kind="ExternalOutput")

    with tile.TileContext(nc) as tc:
        with (
            tc.tile_pool(name="cst", bufs=1) as cst,
            tc.tile_pool(name="pmm", bufs=2, space="PSUM") as pmm,
            tc.tile_pool(name="pout", bufs=1, space="PSUM") as pout,
        ):
            p128 = cst.tile([128, _P128_LEN], f32)
            nc.sync.dma_start(out=p128, in_=d_p128[:])
            pe_p = cst.tile([128, 2 * T + 2 * NT], f32)
            nc.sync.dma_start(out=pe_p, in_=d_pe[:])
            w2tt = cst.tile([128, NT * IN_D], f32)
            nc.sync.dma_start(out=w2tt, in_=d_w2t[:])
            p100 = cst.tile([IN_D, _P100_LEN + 1], f32)
            nc.sync.dma_start(out=p100, in_=d_p100[:])
            wout = cst.tile([HSL, OUT_PAD], f32)
            nc.scalar.dma_start(out=wout, in_=d_wout[:])

            wb = p128[:, _P128_WB:_P128_BGC]
            bgc = p128[:, _P128_BGC:_P128_LEN]
            posb = pe_p[:, 0:T]
            w1r = pe_p[:, 2 * T : 2 * T + NT]
            b1r = pe_p[:, 2 * T + NT : 2 * T + 2 * NT]
            w2t = w2tt.rearrange("p (n k) -> p n k", n=NT)

            tsT = p100[:, _P100_XT:_P100_WP1]
            b2c = p100[:, _P100_LEN : _P100_LEN + 1]
            wp1 = p100[:, _P100_WP1:_P100_BP1]
            bp1 = p100[:, _P100_BP1:_P100_WG]
            wgt = {}
            for gi, nm in enumerate(("g1", "c1", "g2", "c2")):
                o = _P100_WG + gi * NKC * HSL
                wgt[nm] = p100[:, o : o + NKC * HSL].rearrange(
                    "p (n m) -> p n m", n=NKC
                )

            # pe stage: peT[h, t] = relu(pos_t*w1[h]+b1[h]); pos_embT = sum_h
            peT = cst.tile([128, NT, T], f32)
            for i in range(NT):
                nc.scalar.activation(
                    out=peT[:, i, :],
                    in_=posb,
                    func=AF.Relu,
                    bias=b1r[:, i : i + 1],
                    scale=w1r[:, i : i + 1],
                )
            ppe = pmm.tile([IN_D, T], f32, tag="mm")
            for i in range(NT):
                nc.tensor.matmul(
                    ppe, w2t[:, i, :], peT[:, i, :], start=(i == 0), stop=(i == NT - 1)
                )
            xT = cst.tile([IN_D, T], f32)
            nc.vector.scalar_tensor_tensor(
                out=xT, in0=ppe, scalar=b2c[:, 0:1], in1=tsT, op0=OP.add, op1=OP.add
            )

            _build_core(
                nc, tile, mybir, tc, cst, pmm, pout,
                xT, wp1, bp1, wgt, bgc, wb, wout, d_out,
            )

    nc.finalize()
    return nc


def _prep_common(inputs):
    f = np.float32
    arr = {k: np.asarray(v, dtype=f) for k, v in inputs.items() if k != "positions"}
    pos = np.asarray(inputs["positions"]).astype(f)
    ts = arr["time_steps"]
    S = ts.shape[0]
    # softmax(arange(S,0,-1))[t] = exp(-t)/Z with Z the geometric sum.
    Z = (1.0 - np.exp(-float(S))) / (1.0 - np.exp(-1.0))
    w = (np.exp(-np.arange(T, dtype=np.float64)) / Z).astype(f)
    return arr, pos, w


def _core_p128(a, p128_base, sl):
    pc = p128_base.copy()
    pc[:, _P128_BGC + 0] = a["b_g1"][sl]
    pc[:, _P128_BGC + 1] = a["b_c1"][sl]
    pc[:, _P128_BGC + 2] = a["b_g2"][sl]
    pc[:, _P128_BGC + 3] = a["b_c2"][sl]
    return pc


def _core_wg(a, sl):
    wg = np.zeros((IN_D, _WG_LEN), np.float32)
    for gi, k in enumerate(("W_g1", "W_c1", "W_g2", "W_c2")):
        o = gi * NKC * HSL
        blk = a[k][sl].T.reshape(NKC, 100, HSL).transpose(1, 0, 2)
        wg[:, o : o + NKC * HSL] = blk.reshape(100, NKC * HSL)
    return wg


def _core_wout(a, sl, dtype=np.float16):
    wo = np.zeros((HSL, OUT_PAD), dtype)
    wo[:, :OUT_D] = a["W_out"][:, sl].T.astype(dtype)
    return wo


def _prep_inputs(inputs):
    """Host-side shard/layout prep. Returns (mode, per-core input maps, b_out)."""
    a, pos, w = _prep_common(inputs)
    ts = a["time_steps"]

    p128 = np.zeros((128, _P128_LEN), np.float32)
    p128[:, _P128_WB:_P128_BGC] = w[None, :]

    fast = bool((a["b_pe1"] == 0).all() and (pos[:T] >= 0).all())
    if fast:
        # rank-1 pos_emb folded into xT (see module docstring)
        v = a["W_pe2"] @ np.maximum(a["W_pe1"][:, 0], 0.0)
        xT = ts[:T].T + v[:, None] * pos[None, :T] + a["b_pe2"][:, None]
        # fold proj1 into the gate weights/biases (linear-layer composition)
        Wf = {k: a[k] @ a["W_p1"] for k in ("W_g1", "W_c1", "W_g2", "W_c2")}
        bf = {
            "b_g1": a["b_g1"] + a["W_g1"] @ a["b_p1"],
            "b_c1": a["b_c1"] + a["W_c1"] @ a["b_p1"],
            "b_g2": a["b_g2"] + a["W_g2"] @ a["b_p1"],
            "b_c2": a["b_c2"] + a["W_c2"] @ a["b_p1"],
        }
        in_maps = []
        for ci in range(NCORES):
            sl = slice(ci * HSL, (ci + 1) * HSL)
            pa1 = np.zeros((128, _P1_LEN), np.float32)
            h16 = pa1.view(np.float16)
            h16[:IN_D, 0 : T] = xT.astype(np.float16)
            h16[IN_D, 0 : T] = 1.0
            h16[IN_D + 1, T] = 1.0  # zero-gate reset column flag
            for gi, k in enumerate(("W_g1", "W_c1", "W_g2", "W_c2")):
                o = 2 * _P1_WGF + gi * HSL
                h16[:IN_D, o : o + HSL] = Wf[k][sl].T.astype(np.float16)
                h16[IN_D, o : o + HSL] = bf["b" + k[1:]][sl].astype(np.float16)
                # row 101 meets the reset column's flag: -30 drives the
                # gate sigmoid to 0; 0 keeps the candidate tanh at 0.
                h16[IN_D + 1, o : o + HSL] = -30.0 if k[2] == "g" else 0.0
            w16 = w.astype(np.float16)
            h16[:, 2 * _P1_WB : 2 * _P1_WB + T] = w16[None, :]
            h16[:, 2 * _P1_WB + T + 1 : 2 * _P1_WB + TC] = w16[None, :]
            import ml_dtypes
            wt = a["W_out"][:, sl].T  # (HSL, OUT_D)
            wo = np.zeros((HSL, WB_BYTES), np.uint8)
            wo[:, : 2 * C16] = np.ascontiguousarray(
                wt[:, :C16].astype(np.float16)).view(np.uint8)
            wo[:, 2 * C16 : 2 * C16 + C8] = np.ascontiguousarray(
                wt[:, C16:].astype(ml_dtypes.float8_e4m3)).view(np.uint8)
            in_maps.append({
                "pack1": pa1,
                "wout_t": wo,
            })
        return "fast", in_maps, a["b_out"]

    # general fallback: pe stage on device
    pe_p = np.zeros((128, 2 * T + 2 * NT), np.float32)
    pe_p[:, 0:T] = pos[None, :T]
    pe_p[:, 2 * T : 2 * T + NT] = a["W_pe1"][:, 0].reshape(NT, 128).T
    pe_p[:, 2 * T + NT : 2 * T + 2 * NT] = a["b_pe1"].reshape(NT, 128).T
    w2t = (
        a["W_pe2"].T.reshape(NT, 128, IN_D).transpose(1, 0, 2).reshape(128, NT * IN_D)
    ).copy()
    p100 = np.zeros((IN_D, _P100_LEN + 1), np.float32)
    p100[:, _P100_XT:_P100_WP1] = ts[:T].T
    p100[:, _P100_WP1:_P100_BP1] = a["W_p1"].T
    p100[:, _P100_BP1:_P100_WG] = a["b_p1"].reshape(NKC, 100).T
    p100[:, _P100_LEN] = a["b_pe2"]
    in_maps = []
    for ci in range(NCORES):
        sl = slice(ci * HSL, (ci + 1) * HSL)
        full = p100.copy()
        full[:, _P100_WG:_P100_LEN] = _core_wg(a, sl)
        in_maps.append({
            "pack128": _core_p128(a, p128, sl),
            "pack100": full,
            "pe_pack": pe_p,
            "w2t": w2t,
            "wout_t": _core_wout(a, sl, dtype=np.float32),
        })
    return "general", in_maps, a["b_out"]


def _run(inputs, trace=False):
    from concourse.bass_utils import run_bass_kernel_spmd

    mode, in_maps, b_out = _prep_inputs(inputs)
    key = f"nc_{mode}"
    if key not in _CACHE:
        _CACHE[key] = _build_nc_fast() if mode == "fast" else _build_nc_general()
    nc = _CACHE[key]
    res = run_bass_kernel_spmd(nc, in_maps, core_ids=list(range(NCORES)), trace=trace)
    acc = np.zeros(OUT_D, dtype=np.float32)
    for r in res.results:
        part = np.asarray(r["out_part"], dtype=np.float32).reshape(128, NJ)
        if mode == "fast":
            p = np.zeros(OUT_D, np.float32)
            p[: 128 * (NJ - 1)] = part[:, : NJ - 1].T.ravel()
            p[OUT_D - 128 :] = part[:, NJ - 1]
            acc = acc + p
        else:
            acc = acc + part.T.ravel()[:OUT_D]
    return (acc + b_out).astype(np.float32), res


def kernel(**inputs):
    out, _ = _run(inputs, trace=False)
    return out


# revision 23
# speedup vs baseline: 1.0204x; 1.0013x over previous
"""Trainium2 Bass kernel for nn_EnhancedGatedTemporalFusion.

Mathematical structure exploited (all exact at f32 precision):
  * The self-attention block in the reference is dead code (its result is
    never used downstream), so it is skipped.
  * The output weighting is softmax(arange(S,0,-1)), i.e. w[t] = exp(-t)/Z.
    Since |outputs[t]| <= 2 (convex combinations of tanh values), the tail
    sum over t >= T is bounded by 2*e^{-T}; at T=16 that is ~2e-7 absolute
    against a result of magnitude ~0.05 — far below the fp16 weight
    quantization noise already present.
  * The gated update h' = g*h + (1-g)*c is an affine recurrence that maps
    1:1 onto the DVE TensorTensorScanArith instruction.
  * When b_pe1 == 0 and positions >= 0 (true for this problem's inputs),
    relu(pos_t*w1[h]) = pos_t*relu(w1[h]), so the positional-encoding MLP
    is rank-1: pos_emb[t, :] = pos_t * (W_pe2 @ relu(W_pe1[:, 0])).  That
    O(T*IN_D) correction is folded into the x input on the host, removing
    the whole 1024-wide pe stage from the device graph.  A general device
    path is kept as a fallback and used automatically if the guard fails.

Sharding across the 8 cores: the hidden dim H=1024 is split 128 lanes per
core.  Each core computes its own h-slice of the gate/candidate GEMMs, the
scan, the exp-weighted time reduction, and a partial product of the final
H->2613 projection over its h-slice.  The 8 partial vectors are summed on
the host (contraction unshard) and the output bias is added.

Timeline-critical layout (per core):
  * One small front DMA (HWDGE on SP) carries the fp16-packed xT, fused
    gate weights and softmax weights; the big W_out slice rides a second
    SWDGE DMA whose descriptor generation overlaps the front DMA's issue,
    so its transfer starts the moment the pack transfer ends.
  * W_out is byte-packed mixed-precision: the first NJ-NJ8 GEMV tiles are
    fp16, the last NJ8 tiles are fp8e4m3, halving those columns' DMA
    bytes.  The measured end-to-end relative error of this split is
    ~1.88e-2 against the 2e-2 gate (the error is dominated by the fp8
    block; fp8 on any "full-information" tensor costs ~3.3% rms, so only
    a sqrt(C8/OUT_D) fraction is affordable).  The PE accepts the mixed
    (fp8 lhsT x fp16 rhs) matmuls directly.
  * The (128, 21) partial-output store is a kv_writeback prepared
    (descriptor-generated) right after the W_out descriptor job, early in
    the kernel, and fired by a trigger_dma when the data is ready.  The
    trigger only costs the sequencer dispatch + transfer + completion
    propagation, skipping the whole HWDGE issue + DGE->DMA delay chain
    (~1.3us) that a plain dma_start would put after the last compute.
"""

import sys

import numpy as np

if "/opt/trn_rl_repo" not in sys.path:
    sys.path.insert(0, "/opt/trn_rl_repo")

T = 16           # truncated horizon (tail < 2e-7 of the result, see above)
IN_D = 100       # input_dim
CH = 300         # proj1 out dim
H = 1024         # hidden dim
OUT_D = 2613     # output dim
OUT_PAD = 2688   # 21 * 128
NJ = OUT_PAD // 128
NCORES = 8
HSL = H // NCORES      # h-lanes per core
# mixed-precision W_out: the last NJ8 GEMV tiles use fp8e4m3 weights.  The
# measured end-to-end error for this split is ~1.88e-2 (gate: 2e-2); each
# fp8 column halves its DMA bytes.  Byte layout per partition row:
# [C16 fp16 cols | C8 fp8 cols].
NJ8 = 7
C16 = 128 * (NJ - NJ8)          # fp16 columns
C8 = OUT_D - C16                # fp8 columns
WB_BYTES = 2 * C16 + C8 + 1     # packed row bytes (+1 pad byte: even row)
NT = H // 128          # h tiles of 128 for the fallback pe stage
NKC = CH // 100        # contraction chunks of 100 for the 300-dim

# pack128 free-dim offsets (wb + per-core gate biases) -- general path
_P128_WB = 0
_P128_BGC = _P128_WB + T
_P128_LEN = _P128_BGC + 4

# fast path: pack1 carries fp16 [xT | ones-row | zc-flag row] (102, T+1),
# fp16 augmented gate weights (102, 4*HSL) with the fused biases in row 100
# (K=102 matmul adds them) and a zero-gate flag in row 101 (-30 pre-act for
# the gates, 0 for the candidates, at the reset column T), and fp16 softmax
# weights laid out [w | 0 | w] (128, 2T+1).  The reset column lets ONE
# TensorTensorScan run both gate pairs back-to-back: h[T] = 0*h[T-1] - 0.
TC = 2 * T + 1
_P1_XT = 0
_P1_WGF = _P1_XT + (T + 2) // 2
_P1_WB = _P1_WGF + 4 * HSL // 2
_P1_LEN = _P1_WB + (TC + 1) // 2

# general-path offsets
_P100_XT = 0
_P100_WP1 = _P100_XT + T
_P100_BP1 = _P100_WP1 + CH
_P100A_LEN = _P100_BP1 + NKC
_WG_LEN = 4 * NKC * HSL
_P100_WG = _P100A_LEN
_P100_LEN = _P100_WG + _WG_LEN

_CACHE = {}


def _build_core(nc, tile, mybir, tc, cst, pmm, pout, xT, wp1, bp1, wgt, bgc, wb, wout, d_out):
    """Shared back end (general path): combined -> gates -> scan -> weighted
    sum -> partial out."""
    f32 = mybir.dt.float32
    AF = mybir.ActivationFunctionType
    OP = mybir.AluOpType

    # combinedT (ch=300 in 3 chunks of 100, t)
    combT = cst.tile([100, NKC, T], f32)
    for ch in range(NKC):
        pcomb = pmm.tile([100, T], f32, tag="mm", name=f"pcomb{ch}")
        nc.tensor.matmul(
            pcomb, wp1[:, ch * 100 : (ch + 1) * 100], xT, start=True, stop=True
        )
        nc.vector.tensor_scalar_add(combT[:, ch, :], pcomb, bp1[:, ch : ch + 1])

    def gate(nm, func, bcol):
        pg = pmm.tile([HSL, T], f32, tag="mm", name=f"p_{nm}")
        for ch in range(NKC):
            nc.tensor.matmul(
                pg,
                wgt[nm][:, ch, :],
                combT[:, ch, :],
                start=(ch == 0),
                stop=(ch == NKC - 1),
            )
        sb = cst.tile([HSL, T], f32, name=f"s_{nm}")
        nc.scalar.activation(
            out=sb, in_=pg, func=func, bias=bgc[:, bcol : bcol + 1], scale=1.0
        )
        return sb

    def upd(g, c, nm):
        u = cst.tile([HSL, T], f32, name=f"u_{nm}")
        nc.vector.tensor_mul(u, g, c)
        nc.vector.tensor_sub(u, c, u)
        h = cst.tile([HSL, T], f32, name=f"h_{nm}")
        nc.vector.tensor_tensor_scan(
            out=h, data0=g, data1=u, initial=0.0, op0=OP.mult, op1=OP.add
        )
        return h

    g1 = gate("g1", AF.Sigmoid, 0)
    c1 = gate("c1", AF.Tanh, 1)
    h1 = upd(g1, c1, "1")
    g2 = gate("g2", AF.Sigmoid, 2)
    c2 = gate("c2", AF.Tanh, 3)
    h2 = upd(g2, c2, "2")

    outs = cst.tile([HSL, T], f32)
    nc.vector.tensor_add(outs, h1, h2)

    # weighted time reduction: wsum[h] = sum_t outs[h,t]*w[t]
    scr = cst.tile([HSL, T], f32)
    nc.vector.tensor_mul(scr, outs, wb)
    wsum = cst.tile([HSL, 1], f32)
    nc.vector.tensor_reduce(out=wsum, in_=scr, axis=mybir.AxisListType.X, op=OP.add)

    # partial final projection, d on partitions: out[p, j] = out_d, d=128j+p
    po = pout.tile([128, NJ], f32)
    for j in range(NJ):
        nc.tensor.matmul(
            po[:, j : j + 1],
            wout[:, j * 128 : (j + 1) * 128],
            wsum,
            start=True,
            stop=True,
        )
    ob = cst.tile([128, NJ], f32)
    nc.vector.tensor_copy(ob, po)
    nc.sync.dma_start(out=d_out[:], in_=ob)


def _build_nc_fast():
    """Fast path: pos_emb folded into xT on the host.

    Raw bass (no TileContext): every cross-engine edge is a manual
    semaphore.  This enables the SWDGE prepare/trigger idiom for the final
    store (Tile's managed path deadlocks on the deferred DMASW lane sem in
    this snapshot) and drops the framework's per-instruction bookkeeping.

    Semaphore protocol (all start at 0):
      sD    DVE init memsets: dummy (>=1), zb (>=2), kvidx (>=3)
      sP    pack1 DMA complete (+16)
      sW    W_out DMA complete (+16)
      sG2   gate matmuls drained to PSUM (+1)
      sA    activations done: gp (+1), cp (+1)
      sV    DVE chain ticks: up (+1), hp (+1), scrp (+1)
      sR    wsum16 reduced (+1)
      sPo   GEMV drained to PSUM (+1)
      sOb   ob copied PSUM->SBUF (+1)
      sPre  kv_writeback descriptors written to the SWDGE ring (+1)
      sKV   kv_writeback DMA complete (+16, baked into the descriptors)
    """
    import concourse.bacc as bacc
    from concourse import mybir

    f32 = mybir.dt.float32
    f16 = mybir.dt.float16
    i32 = mybir.dt.int32
    AF = mybir.ActivationFunctionType
    OP = mybir.AluOpType

    nc = bacc.Bacc("TRN2", target_bir_lowering=False, debug=False)

    d_pA1 = nc.dram_tensor("pack1", [128, _P1_LEN], f32, kind="ExternalInput")
    d_wout = nc.dram_tensor("wout_t", [HSL, WB_BYTES], mybir.dt.uint8, kind="ExternalInput")
    # kv_writeback-shaped output: [batch=1, dhi=128, dho=1, n_ctx=NJ];
    # element (0, p, 0, j) holds out_d for d = 128*j + p.
    d_out = nc.dram_tensor("out_part", [1, 128, 1, NJ], f32, kind="ExternalOutput")

    sD = nc.alloc_semaphore("sD")
    sP = nc.alloc_semaphore("sP")
    sW = nc.alloc_semaphore("sW")
    sG2 = nc.alloc_semaphore("sG2")
    sA = nc.alloc_semaphore("sA")
    sV = nc.alloc_semaphore("sV")
    sR = nc.alloc_semaphore("sR")
    sPo = nc.alloc_semaphore("sPo")
    sOb = nc.alloc_semaphore("sOb")
    sPre = nc.alloc_semaphore("sPre")
    sKV = nc.alloc_semaphore("sKV")

    # SBUF / PSUM allocations
    pA1 = nc.alloc_sbuf_tensor("k_pA1", [128, _P1_LEN], f32)
    wout = nc.alloc_sbuf_tensor("k_wout", [HSL, WB_BYTES], mybir.dt.uint8)
    zb = nc.alloc_sbuf_tensor("k_zb", [128, 1], f32)
    kvidx = nc.alloc_sbuf_tensor("k_kvidx", [128, 1], i32)
    dummy = nc.alloc_sbuf_tensor("k_dummy", [1, 1], f32)
    gp = nc.alloc_sbuf_tensor("k_gp", [HSL, TC], f16)
    cp = nc.alloc_sbuf_tensor("k_cp", [HSL, TC], f16)
    up = nc.alloc_sbuf_tensor("k_up", [HSL, TC], f16)
    hp = nc.alloc_sbuf_tensor("k_hp", [HSL, TC], f16)
    scrp = nc.alloc_sbuf_tensor("k_scrp", [HSL, TC], f16)
    wsum16 = nc.alloc_sbuf_tensor("k_wsum16", [HSL, 1], f16)
    ob = nc.alloc_sbuf_tensor("k_ob", [128, NJ], f32)
    pwarm = nc.alloc_psum_tensor("k_pwarm", [1, 1], f32)
    pgg = nc.alloc_psum_tensor("k_pgg", [HSL, TC], f32)
    pcc = nc.alloc_psum_tensor("k_pcc", [HSL, TC], f32)
    po = nc.alloc_psum_tensor("k_po", [128, NJ], f32)

    xTw = pA1[0 : IN_D + 2, _P1_XT:_P1_WGF].bitcast(f16)
    wgf = pA1[0 : IN_D + 2, _P1_WGF:_P1_WB].bitcast(f16)
    wbp = pA1[:, _P1_WB:_P1_LEN].bitcast(f16)

    # --- SP: the front DMA; nothing else ---
    nc.sync.dma_start(out=pA1[:], in_=d_pA1[:]).then_inc(sP, 16)

    # --- Pool: W_out DMA, then the prepared final store ---
    # SWDGE desc-gen for W_out runs during pack1's HWDGE issue window, so
    # its transfer starts right as pack1's transfer drains.
    nc.gpsimd.dma_start(out=wout[:], in_=d_wout[:]).then_inc(sW, 16)
    nc.gpsimd.wait_ge(sD, 3)  # kvidx ready (read at desc-gen time)
    nc.gpsimd.kv_writeback(
        d_out[:],
        ob[:].rearrange("p (a b n) -> p a b n", a=1, b=1),
        kvidx[:],
        prepare_only=True,
        sem=sKV,
    ).then_inc(sPre, 1)
    nc.gpsimd.wait_ge(sPre, 1)  # descriptors in the ring (early, cheap)
    # the data-ready wait rides on the trigger itself, saving a separate
    # EventSemaphore's sequencer cost on the critical tail.
    nc.gpsimd.trigger_dma(count=1)._wait_ge(sOb, 1)

    # --- SP: the kernel-end gate.  SP's semaphore receive overhead is 0
    # and its sequencer decode is the cheapest, so the final wait on the
    # store-completion sem resolves earlier there than on Pool. ---
    nc.sync.wait_ge(sKV, 16)  # final store complete

    # --- DVE: init memsets, then the scan chain ---
    nc.vector.memset(dummy[:], 0.0).then_inc(sD, 1)
    nc.vector.memset(zb[:], 0.0).then_inc(sD, 1)
    nc.vector.memset(kvidx[:], 0).then_inc(sD, 1)
    nc.vector.wait_ge(sA, 2)
    # The DVE engine is freed before its SBUF write-ack returns, so every
    # same-engine RAW edge in this chain needs an explicit tick (sV) -- the
    # sem fires only after the ack, like Tile's engine-tick waits.
    # negu = (g-1)*c in one DVE op; the scan then computes
    # h = g*h - negu = g*h + (1-g)*c directly (op1=subtract).  At the reset
    # column T: g=0, c=0 => h[T]=0, so one scan covers both gate pairs.
    nc.vector.scalar_tensor_tensor(
        out=up[:], in0=gp[:], scalar=1.0, in1=cp[:],
        op0=OP.subtract, op1=OP.mult,
    ).then_inc(sV, 1)
    nc.vector.wait_ge(sV, 1)
    nc.vector.tensor_tensor_scan(
        out=hp[:], data0=gp[:], data1=up[:],
        initial=0.0, op0=OP.mult, op1=OP.subtract,
    ).then_inc(sV, 1)
    nc.vector.wait_ge(sV, 2)
    # wsum[h] = sum_t (h1+h2)*w == flat weighted sum over both halves
    nc.vector.tensor_mul(scrp[:], hp[:], wbp[:, 0:TC]).then_inc(sV, 1)
    nc.vector.wait_ge(sV, 3)
    with nc.allow_low_precision("33-term f32-accumulated reduce, fp16 store"):
        nc.vector.tensor_reduce(
            out=wsum16[:],
            in_=scrp[:],
            axis=mybir.AxisListType.X,
            op=OP.add,
        ).then_inc(sR, 1)
    nc.vector.wait_ge(sPo, 1)
    nc.vector.tensor_copy(ob[:], po[:]).then_inc(sOb, 1)

    # --- ACT: table-load hoist, then the two activations ---
    # dummy sigmoid right at the start makes bacc emit the (1.3us) ACT
    # table load during the input-DMA shadow.
    nc.scalar.wait_ge(sD, 1)   # dummy scratch ready
    nc.scalar.activation(out=dummy[:], in_=dummy[:], func=AF.Sigmoid)
    nc.scalar.wait_ge(sD, 2)   # zb (bias) ready
    nc.scalar.wait_ge(sG2, 1)  # gate matmuls drained
    nc.scalar.activation(
        out=gp[:], in_=pgg[:], func=AF.Sigmoid, bias=zb[:, 0:1]
    ).then_inc(sA, 1)
    nc.scalar.activation(
        out=cp[:], in_=pcc[:], func=AF.Tanh, bias=zb[:, 0:1]
    ).then_inc(sA, 1)

    # --- PE: warmup, gate matmuls, final GEMV ---
    # p-state warmup during the DMA shadow (on the zeroed zb scratch).
    nc.tensor.wait_ge(sD, 2)
    for _ in range(8):
        nc.tensor.matmul(pwarm[:], zb[0:1, 0:1], zb[0:1, 0:1], start=True, stop=True)
    nc.tensor.wait_ge(sP, 16)
    # paired gates: three matmuls per PSUM tile (pair1, reset col, pair2),
    # one activation over the whole row (biases ride in wgf row 100 against
    # the ones-row of xTw; the reset column's -30/0 rides row 101).
    nc.tensor.matmul(pgg[:, 0:T], wgf[:, 0 * HSL : 1 * HSL], xTw[:, 0:T],
                     start=True, stop=True)
    nc.tensor.matmul(pgg[:, T : T + 1], wgf[:, 0 * HSL : 1 * HSL],
                     xTw[:, T : T + 1], start=True, stop=True)
    nc.tensor.matmul(pgg[:, T + 1 : TC], wgf[:, 2 * HSL : 3 * HSL], xTw[:, 0:T],
                     start=True, stop=True)
    nc.tensor.matmul(pcc[:, 0:T], wgf[:, 1 * HSL : 2 * HSL], xTw[:, 0:T],
                     start=True, stop=True)
    nc.tensor.matmul(pcc[:, T : T + 1], wgf[:, 1 * HSL : 2 * HSL],
                     xTw[:, T : T + 1], start=True, stop=True)
    nc.tensor.matmul(pcc[:, T + 1 : TC], wgf[:, 3 * HSL : 4 * HSL], xTw[:, 0:T],
                     start=True, stop=True)
    nc.tensor.drain().then_inc(sG2, 1)
    nc.tensor.wait_ge(sR, 1)
    nc.tensor.wait_ge(sW, 16)
    w16v = wout[:, 0 : 2 * C16].bitcast(f16)
    w8v = wout[:, 2 * C16 : 2 * C16 + C8].bitcast(mybir.dt.float8e4)
    for j in range(NJ):
        # the last tile overlaps the previous one (cols OUT_D-128..OUT_D)
        # so every PSUM row is written with M=128; the host gather maps
        # column 20 to d = OUT_D-128+p.
        d0 = min(j * 128, OUT_D - 128)
        lhsT = w16v[:, d0 : d0 + 128] if j < NJ - NJ8 else w8v[:, d0 - C16 : d0 - C16 + 128]
        nc.tensor.matmul(
            po[:, j : j + 1],
            lhsT,
            wsum16[:],
            start=True,
            stop=True,
        )
    nc.tensor.drain().then_inc(sPo, 1)

    nc.finalize()
    return nc


def _build_nc_general():
    """Fallback: full pe stage on device (used when the rank-1 guard fails)."""
    import concourse.bacc as bacc
    import concourse.tile as tile
    from concourse import mybir

    f32 = mybir.dt.float32
    AF = mybir.ActivationFunctionType
    OP = mybir.AluOpType

    nc = bacc.Bacc("TRN2", target_bir_lowering=False, debug=False)

    d_p128 = nc.dram_tensor("pack128", [128, _P128_LEN], f32, kind="ExternalInput")
    d_pe = nc.dram_tensor("pe_pack", [128, 2 * T + 2 * NT], f32, kind="ExternalInput")
    d_w2t = nc.dram_tensor("w2t", [128, NT * IN_D], f32, kind="ExternalInput")
    d_p100 = nc.dram_tensor("pack100", [IN_D, _P100_LEN + 1], f32, kind="ExternalInput")
    d_wout = nc.dram_tensor("wout_t", [HSL, OUT_PAD], f32, kind="ExternalInput")
    d_out = nc.dram_tensor("out_part", [128, NJ], f32, kind="ExternalOutput")

    with tile.TileContext(nc) as tc:
        with (
            tc.tile_pool(name="cst", bufs=1) as cst,
            tc.tile_pool(name="pmm", bufs=2, space="PSUM") as pmm,
            tc.tile_pool(name="pout", bufs=1, space="PSUM") as pout,
        ):
            p128 = cst.tile([128, _P128_LEN], f32)
            nc.sync.dma_start(out=p128, in_=d_p128[:])
            pe_p = cst.tile([128, 2 * T + 2 * NT], f32)
            nc.sync.dma_start(out=pe_p, in_=d_pe[:])
            w2tt = cst.tile([128, NT * IN_D], f32)
            nc.sync.dma_start(out=w2tt, in_=d_w2t[:])
            p100 = cst.tile([IN_D, _P100_LEN + 1], f32)
            nc.sync.dma_start(out=p100, in_=d_p100[:])
            wout = cst.tile([HSL, OUT_PAD], f32)
            nc.scalar.dma_start(out=wout, in_=d_wout[:])

            wb = p128[:, _P128_WB:_P128_BGC]
            bgc = p128[:, _P128_BGC:_P128_LEN]
            posb = pe_p[:, 0:T]
            w1r = pe_p[:, 2 * T : 2 * T + NT]
            b1r = pe_p[:, 2 * T + NT : 2 * T + 2 * NT]
            w2t = w2tt.rearrange("p (n k) -> p n k", n=NT)

            tsT = p100[:, _P100_XT:_P100_WP1]
            b2c = p100[:, _P100_LEN : _P100_LEN + 1]
            wp1 = p100[:, _P100_WP1:_P100_BP1]
            bp1 = p100[:, _P100_BP1:_P100_WG]
            wgt = {}
            for gi, nm in enumerate(("g1", "c1", "g2", "c2")):
                o = _P100_WG + gi * NKC * HSL
                wgt[nm] = p100[:, o : o + NKC * HSL].rearrange(
                    "p (n m) -> p n m", n=NKC
                )

            # pe stage: peT[h, t] = relu(pos_t*w1[h]+b1[h]); pos_embT = sum_h
            peT = cst.tile([128, NT, T], f32)
            for i in range(NT):
                nc.scalar.activation(
                    out=peT[:, i, :],
                    in_=posb,
                    func=AF.Relu,
                    bias=b1r[:, i : i + 1],
                    scale=w1r[:, i : i + 1],
                )
            ppe = pmm.tile([IN_D, T], f32, tag="mm")
            for i in range(NT):
                nc.tensor.matmul(
                    ppe, w2t[:, i, :], peT[:, i, :], start=(i == 0), stop=(i == NT - 1)
                )
            xT = cst.tile([IN_D, T], f32)
            nc.vector.scalar_tensor_tensor(
                out=xT, in0=ppe, scalar=b2c[:, 0:1], in1=tsT, op0=OP.add, op1=OP.add
            )

            _build_core(
                nc, tile, mybir, tc, cst, pmm, pout,
                xT, wp1, bp1, wgt, bgc, wb, wout, d_out,
            )

    nc.finalize()
    return nc


def _prep_common(inputs):
    f = np.float32
    arr = {k: np.asarray(v, dtype=f) for k, v in inputs.items() if k != "positions"}
    pos = np.asarray(inputs["positions"]).astype(f)
    ts = arr["time_steps"]
    S = ts.shape[0]
    # softmax(arange(S,0,-1))[t] = exp(-t)/Z with Z the geometric sum.
    Z = (1.0 - np.exp(-float(S))) / (1.0 - np.exp(-1.0))
    w = (np.exp(-np.arange(T, dtype=np.float64)) / Z).astype(f)
    return arr, pos, w


def _core_p128(a, p128_base, sl):
    pc = p128_base.copy()
    pc[:, _P128_BGC + 0] = a["b_g1"][sl]
    pc[:, _P128_BGC + 1] = a["b_c1"][sl]
    pc[:, _P128_BGC + 2] = a["b_g2"][sl]
    pc[:, _P128_BGC + 3] = a["b_c2"][sl]
    return pc


def _core_wg(a, sl):
    wg = np.zeros((IN_D, _WG_LEN), np.float32)
    for gi, k in enumerate(("W_g1", "W_c1", "W_g2", "W_c2")):
        o = gi * NKC * HSL
        blk = a[k][sl].T.reshape(NKC, 100, HSL).transpose(1, 0, 2)
        wg[:, o : o + NKC * HSL] = blk.reshape(100, NKC * HSL)
    return wg


def _core_wout(a, sl, dtype=np.float16):
    wo = np.zeros((HSL, OUT_PAD), dtype)
    wo[:, :OUT_D] = a["W_out"][:, sl].T.astype(dtype)
    return wo


def _prep_inputs(inputs):
    """Host-side shard/layout prep. Returns (mode, per-core input maps, b_out)."""
    a, pos, w = _prep_common(inputs)
    ts = a["time_steps"]

    p128 = np.zeros((128, _P128_LEN), np.float32)
    p128[:, _P128_WB:_P128_BGC] = w[None, :]

    fast = bool((a["b_pe1"] == 0).all() and (pos[:T] >= 0).all())
    if fast:
        # rank-1 pos_emb folded into xT (see module docstring)
        v = a["W_pe2"] @ np.maximum(a["W_pe1"][:, 0], 0.0)
        xT = ts[:T].T + v[:, None] * pos[None, :T] + a["b_pe2"][:, None]
        # fold proj1 into the gate weights/biases (linear-layer composition)
        Wf = {k: a[k] @ a["W_p1"] for k in ("W_g1", "W_c1", "W_g2", "W_c2")}
        bf = {
            "b_g1": a["b_g1"] + a["W_g1"] @ a["b_p1"],
            "b_c1": a["b_c1"] + a["W_c1"] @ a["b_p1"],
            "b_g2": a["b_g2"] + a["W_g2"] @ a["b_p1"],
            "b_c2": a["b_c2"] + a["W_c2"] @ a["b_p1"],
        }
        in_maps = []
        for ci in range(NCORES):
            sl = slice(ci * HSL, (ci + 1) * HSL)
            pa1 = np.zeros((128, _P1_LEN), np.float32)
            h16 = pa1.view(np.float16)
            h16[:IN_D, 0 : T] = xT.astype(np.float16)
            h16[IN_D, 0 : T] = 1.0
            h16[IN_D + 1, T] = 1.0  # zero-gate reset column flag
            for gi, k in enumerate(("W_g1", "W_c1", "W_g2", "W_c2")):
                o = 2 * _P1_WGF + gi * HSL
                h16[:IN_D, o : o + HSL] = Wf[k][sl].T.astype(np.float16)
                h16[IN_D, o : o + HSL] = bf["b" + k[1:]][sl].astype(np.float16)
                # row 101 meets the reset column's flag: -30 drives the
                # gate sigmoid to 0; 0 keeps the candidate tanh at 0.
                h16[IN_D + 1, o : o + HSL] = -30.0 if k[2] == "g" else 0.0
            w16 = w.astype(np.float16)
            h16[:, 2 * _P1_WB : 2 * _P1_WB + T] = w16[None, :]
            h16[:, 2 * _P1_WB + T + 1 : 2 * _P1_WB + TC] = w16[None, :]
            import ml_dtypes
            wt = a["W_out"][:, sl].T  # (HSL, OUT_D)
            wo = np.zeros((HSL, WB_BYTES), np.uint8)
            wo[:, : 2 * C16] = np.ascontiguousarray(
                wt[:, :C16].astype(np.float16)).view(np.uint8)
            wo[:, 2 * C16 : 2 * C16 + C8] = np.ascontiguousarray(
                wt[:, C16:].astype(ml_dtypes.float8_e4m3)).view(np.uint8)
            in_maps.append({
                "pack1": pa1,
                "wout_t": wo,
            })
        return "fast", in_maps, a["b_out"]

    # general fallback: pe stage on device
    pe_p = np.zeros((128, 2 * T + 2 * NT), np.float32)
    pe_p[:, 0:T] = pos[None, :T]
    pe_p[:, 2 * T : 2 * T + NT] = a["W_pe1"][:, 0].reshape(NT, 128).T
    pe_p[:, 2 * T + NT : 2 * T + 2 * NT] = a["b_pe1"].reshape(NT, 128).T
    w2t = (
        a["W_pe2"].T.reshape(NT, 128, IN_D).transpose(1, 0, 2).reshape(128, NT * IN_D)
    ).copy()
    p100 = np.zeros((IN_D, _P100_LEN + 1), np.float32)
    p100[:, _P100_XT:_P100_WP1] = ts[:T].T
    p100[:, _P100_WP1:_P100_BP1] = a["W_p1"].T
    p100[:, _P100_BP1:_P100_WG] = a["b_p1"].reshape(NKC, 100).T
    p100[:, _P100_LEN] = a["b_pe2"]
    in_maps = []
    for ci in range(NCORES):
        sl = slice(ci * HSL, (ci + 1) * HSL)
        full = p100.copy()
        full[:, _P100_WG:_P100_LEN] = _core_wg(a, sl)
        in_maps.append({
            "pack128": _core_p128(a, p128, sl),
            "pack100": full,
            "pe_pack": pe_p,
            "w2t": w2t,
            "wout_t": _core_wout(a, sl, dtype=np.float32),
        })
    return "general", in_maps, a["b_out"]


def _run(inputs, trace=False):
    from concourse.bass_utils import run_bass_kernel_spmd

    mode, in_maps, b_out = _prep_inputs(inputs)
    key = f"nc_{mode}"
    if key not in _CACHE:
        _CACHE[key] = _build_nc_fast() if mode == "fast" else _build_nc_general()
    nc = _CACHE[key]
    res = run_bass_kernel_spmd(nc, in_maps, core_ids=list(range(NCORES)), trace=trace)
    acc = np.zeros(OUT_D, dtype=np.float32)
    for r in res.results:
        part = np.asarray(r["out_part"], dtype=np.float32).reshape(128, NJ)
        if mode == "fast":
            p = np.zeros(OUT_D, np.float32)
            p[: 128 * (NJ - 1)] = part[:, : NJ - 1].T.ravel()
            p[OUT_D - 128 :] = part[:, NJ - 1]
            acc = acc + p
        else:
            acc = acc + part.T.ravel()[:OUT_D]
    return (acc + b_out).astype(np.float32), res


def kernel(**inputs):
    out, _ = _run(inputs, trace=False)
    return out


# revision 24
# speedup vs baseline: 1.0235x; 1.0031x over previous
"""Trainium2 Bass kernel for nn_EnhancedGatedTemporalFusion.

Mathematical structure exploited (all exact at f32 precision):
  * The self-attention block in the reference is dead code (its result is
    never used downstream), so it is skipped.
  * The output weighting is softmax(arange(S,0,-1)), i.e. w[t] = exp(-t)/Z.
    Since |outputs[t]| <= 2 (convex combinations of tanh values), the tail
    sum over t >= T is bounded by 2*e^{-T}; at T=16 that is ~2e-7 absolute
    against a result of magnitude ~0.05 — far below the fp16 weight
    quantization noise already present.
  * The gated update h' = g*h + (1-g)*c is an affine recurrence that maps
    1:1 onto the DVE TensorTensorScanArith instruction.
  * When b_pe1 == 0 and positions >= 0 (true for this problem's inputs),
    relu(pos_t*w1[h]) = pos_t*relu(w1[h]), so the positional-encoding MLP
    is rank-1: pos_emb[t, :] = pos_t * (W_pe2 @ relu(W_pe1[:, 0])).  That
    O(T*IN_D) correction is folded into the x input on the host, removing
    the whole 1024-wide pe stage from the device graph.  A general device
    path is kept as a fallback and used automatically if the guard fails.

Sharding across the 8 cores: the hidden dim H=1024 is split 128 lanes per
core.  Each core computes its own h-slice of the gate/candidate GEMMs, the
scan, the exp-weighted time reduction, and a partial product of the final
H->2613 projection over its h-slice.  The 8 partial vectors are summed on
the host (contraction unshard) and the output bias is added.

Timeline-critical layout (per core):
  * One small front DMA (HWDGE on SP) carries the fp16-packed xT, fused
    gate weights and softmax weights; the big W_out slice rides a second
    SWDGE DMA whose descriptor generation overlaps the front DMA's issue,
    so its transfer starts the moment the pack transfer ends.
  * W_out is byte-packed mixed-precision: the first NJ-NJ8 GEMV tiles are
    fp16, the last NJ8 tiles are fp8e4m3, halving those columns' DMA
    bytes.  The measured end-to-end relative error of this split is
    ~1.88e-2 against the 2e-2 gate (the error is dominated by the fp8
    block; fp8 on any "full-information" tensor costs ~3.3% rms, so only
    a sqrt(C8/OUT_D) fraction is affordable).  The PE accepts the mixed
    (fp8 lhsT x fp16 rhs) matmuls directly.
  * The (128, 21) partial-output store is a kv_writeback prepared
    (descriptor-generated) right after the W_out descriptor job, early in
    the kernel, and fired by a trigger_dma when the data is ready.  The
    trigger only costs the sequencer dispatch + transfer + completion
    propagation, skipping the whole HWDGE issue + DGE->DMA delay chain
    (~1.3us) that a plain dma_start would put after the last compute.
"""

import sys

import numpy as np

if "/opt/trn_rl_repo" not in sys.path:
    sys.path.insert(0, "/opt/trn_rl_repo")

T = 12           # truncated horizon (tail ~4e-5 relative, far below the fp8 noise)
IN_D = 100       # input_dim
CH = 300         # proj1 out dim
H = 1024         # hidden dim
OUT_D = 2613     # output dim
OUT_PAD = 2688   # 21 * 128
NJ = OUT_PAD // 128
NCORES = 8
HSL = H // NCORES      # h-lanes per core
# mixed-precision W_out: the last NJ8 GEMV tiles use fp8e4m3 weights.  The
# measured end-to-end error for this split is ~1.88e-2 (gate: 2e-2); each
# fp8 column halves its DMA bytes.  Byte layout per partition row:
# [C16 fp16 cols | C8 fp8 cols].
NJ8 = 7
C16 = 128 * (NJ - NJ8)          # fp16 columns
C8 = OUT_D - C16                # fp8 columns
WB_BYTES = 2 * C16 + C8 + 1     # packed row bytes (+1 pad byte: even row)
NT = H // 128          # h tiles of 128 for the fallback pe stage
NKC = CH // 100        # contraction chunks of 100 for the 300-dim

# pack128 free-dim offsets (wb + per-core gate biases) -- general path
_P128_WB = 0
_P128_BGC = _P128_WB + T
_P128_LEN = _P128_BGC + 4

# fast path: pack1 carries fp16 [xT | ones-row | zc-flag row] (102, T+1),
# fp16 augmented gate weights (102, 4*HSL) with the fused biases in row 100
# (K=102 matmul adds them) and a zero-gate flag in row 101 (-30 pre-act for
# the gates, 0 for the candidates, at the reset column T), and fp16 softmax
# weights laid out [w | 0 | w] (128, 2T+1).  The reset column lets ONE
# TensorTensorScan run both gate pairs back-to-back: h[T] = 0*h[T-1] - 0.
TC = 2 * T + 1
_P1_XT = 0
_P1_WGF = _P1_XT + (T + 2) // 2
_P1_WB = _P1_WGF + 4 * HSL // 2
_P1_LEN = _P1_WB + (TC + 1) // 2

# general-path offsets
_P100_XT = 0
_P100_WP1 = _P100_XT + T
_P100_BP1 = _P100_WP1 + CH
_P100A_LEN = _P100_BP1 + NKC
_WG_LEN = 4 * NKC * HSL
_P100_WG = _P100A_LEN
_P100_LEN = _P100_WG + _WG_LEN

_CACHE = {}


def _build_core(nc, tile, mybir, tc, cst, pmm, pout, xT, wp1, bp1, wgt, bgc, wb, wout, d_out):
    """Shared back end (general path): combined -> gates -> scan -> weighted
    sum -> partial out."""
    f32 = mybir.dt.float32
    AF = mybir.ActivationFunctionType
    OP = mybir.AluOpType

    # combinedT (ch=300 in 3 chunks of 100, t)
    combT = cst.tile([100, NKC, T], f32)
    for ch in range(NKC):
        pcomb = pmm.tile([100, T], f32, tag="mm", name=f"pcomb{ch}")
        nc.tensor.matmul(
            pcomb, wp1[:, ch * 100 : (ch + 1) * 100], xT, start=True, stop=True
        )
        nc.vector.tensor_scalar_add(combT[:, ch, :], pcomb, bp1[:, ch : ch + 1])

    def gate(nm, func, bcol):
        pg = pmm.tile([HSL, T], f32, tag="mm", name=f"p_{nm}")
        for ch in range(NKC):
            nc.tensor.matmul(
                pg,
                wgt[nm][:, ch, :],
                combT[:, ch, :],
                start=(ch == 0),
                stop=(ch == NKC - 1),
            )
        sb = cst.tile([HSL, T], f32, name=f"s_{nm}")
        nc.scalar.activation(
            out=sb, in_=pg, func=func, bias=bgc[:, bcol : bcol + 1], scale=1.0
        )
        return sb

    def upd(g, c, nm):
        u = cst.tile([HSL, T], f32, name=f"u_{nm}")
        nc.vector.tensor_mul(u, g, c)
        nc.vector.tensor_sub(u, c, u)
        h = cst.tile([HSL, T], f32, name=f"h_{nm}")
        nc.vector.tensor_tensor_scan(
            out=h, data0=g, data1=u, initial=0.0, op0=OP.mult, op1=OP.add
        )
        return h

    g1 = gate("g1", AF.Sigmoid, 0)
    c1 = gate("c1", AF.Tanh, 1)
    h1 = upd(g1, c1, "1")
    g2 = gate("g2", AF.Sigmoid, 2)
    c2 = gate("c2", AF.Tanh, 3)
    h2 = upd(g2, c2, "2")

    outs = cst.tile([HSL, T], f32)
    nc.vector.tensor_add(outs, h1, h2)

    # weighted time reduction: wsum[h] = sum_t outs[h,t]*w[t]
    scr = cst.tile([HSL, T], f32)
    nc.vector.tensor_mul(scr, outs, wb)
    wsum = cst.tile([HSL, 1], f32)
    nc.vector.tensor_reduce(out=wsum, in_=scr, axis=mybir.AxisListType.X, op=OP.add)

    # partial final projection, d on partitions: out[p, j] = out_d, d=128j+p
    po = pout.tile([128, NJ], f32)
    for j in range(NJ):
        nc.tensor.matmul(
            po[:, j : j + 1],
            wout[:, j * 128 : (j + 1) * 128],
            wsum,
            start=True,
            stop=True,
        )
    ob = cst.tile([128, NJ], f32)
    nc.vector.tensor_copy(ob, po)
    nc.sync.dma_start(out=d_out[:], in_=ob)


def _build_nc_fast():
    """Fast path: pos_emb folded into xT on the host.

    Raw bass (no TileContext): every cross-engine edge is a manual
    semaphore.  This enables the SWDGE prepare/trigger idiom for the final
    store (Tile's managed path deadlocks on the deferred DMASW lane sem in
    this snapshot) and drops the framework's per-instruction bookkeeping.

    Semaphore protocol (all start at 0):
      sD    DVE init memsets: dummy (>=1), zb (>=2), kvidx (>=3)
      sP    pack1 DMA complete (+16)
      sW    W_out DMA complete (+16)
      sG2   gate matmuls drained to PSUM (+1)
      sA    activations done: gp (+1), cp (+1)
      sV    DVE chain ticks: up (+1), hp (+1), scrp (+1)
      sR    wsum16 reduced (+1)
      sPo   GEMV drained to PSUM (+1)
      sOb   ob copied PSUM->SBUF (+1)
      sPre  kv_writeback descriptors written to the SWDGE ring (+1)
      sKV   kv_writeback DMA complete (+16, baked into the descriptors)
    """
    import concourse.bacc as bacc
    from concourse import mybir

    f32 = mybir.dt.float32
    f16 = mybir.dt.float16
    i32 = mybir.dt.int32
    AF = mybir.ActivationFunctionType
    OP = mybir.AluOpType

    nc = bacc.Bacc("TRN2", target_bir_lowering=False, debug=False)

    d_pA1 = nc.dram_tensor("pack1", [128, _P1_LEN], f32, kind="ExternalInput")
    d_wout = nc.dram_tensor("wout_t", [HSL, WB_BYTES], mybir.dt.uint8, kind="ExternalInput")
    # kv_writeback-shaped output: [batch=1, dhi=128, dho=1, n_ctx=NJ];
    # element (0, p, 0, j) holds out_d for d = 128*j + p.
    d_out = nc.dram_tensor("out_part", [1, 128, 1, NJ], f32, kind="ExternalOutput")

    sD = nc.alloc_semaphore("sD")
    sP = nc.alloc_semaphore("sP")
    sW = nc.alloc_semaphore("sW")
    sG2 = nc.alloc_semaphore("sG2")
    sA = nc.alloc_semaphore("sA")
    sV = nc.alloc_semaphore("sV")
    sR = nc.alloc_semaphore("sR")
    sPo = nc.alloc_semaphore("sPo")
    sOb = nc.alloc_semaphore("sOb")
    sPre = nc.alloc_semaphore("sPre")
    sKV = nc.alloc_semaphore("sKV")

    # SBUF / PSUM allocations
    pA1 = nc.alloc_sbuf_tensor("k_pA1", [128, _P1_LEN], f32)
    wout = nc.alloc_sbuf_tensor("k_wout", [HSL, WB_BYTES], mybir.dt.uint8)
    zb = nc.alloc_sbuf_tensor("k_zb", [128, 1], f32)
    kvidx = nc.alloc_sbuf_tensor("k_kvidx", [128, 1], i32)
    dummy = nc.alloc_sbuf_tensor("k_dummy", [1, 1], f32)
    gp = nc.alloc_sbuf_tensor("k_gp", [HSL, TC], f16)
    cp = nc.alloc_sbuf_tensor("k_cp", [HSL, TC], f16)
    up = nc.alloc_sbuf_tensor("k_up", [HSL, TC], f16)
    hp = nc.alloc_sbuf_tensor("k_hp", [HSL, TC], f16)
    scrp = nc.alloc_sbuf_tensor("k_scrp", [HSL, TC], f16)
    wsum16 = nc.alloc_sbuf_tensor("k_wsum16", [HSL, 1], f16)
    ob = nc.alloc_sbuf_tensor("k_ob", [128, NJ], f32)
    pwarm = nc.alloc_psum_tensor("k_pwarm", [1, 1], f32)
    pgg = nc.alloc_psum_tensor("k_pgg", [HSL, TC], f32)
    pcc = nc.alloc_psum_tensor("k_pcc", [HSL, TC], f32)
    po = nc.alloc_psum_tensor("k_po", [128, NJ], f32)

    xTw = pA1[0 : IN_D + 2, _P1_XT:_P1_WGF].bitcast(f16)
    wgf = pA1[0 : IN_D + 2, _P1_WGF:_P1_WB].bitcast(f16)
    wbp = pA1[:, _P1_WB:_P1_LEN].bitcast(f16)

    # --- SP: the front DMA; nothing else ---
    nc.sync.dma_start(out=pA1[:], in_=d_pA1[:]).then_inc(sP, 16)

    # --- Pool: W_out DMA, then the prepared final store ---
    # SWDGE desc-gen for W_out runs during pack1's HWDGE issue window, so
    # its transfer starts right as pack1's transfer drains.
    nc.gpsimd.dma_start(out=wout[:], in_=d_wout[:]).then_inc(sW, 16)
    nc.gpsimd.wait_ge(sD, 3)  # kvidx ready (read at desc-gen time)
    nc.gpsimd.kv_writeback(
        d_out[:],
        ob[:].rearrange("p (a b n) -> p a b n", a=1, b=1),
        kvidx[:],
        prepare_only=True,
        sem=sKV,
    ).then_inc(sPre, 1)
    nc.gpsimd.wait_ge(sPre, 1)  # descriptors in the ring (early, cheap)
    # the data-ready wait rides on the trigger itself, saving a separate
    # EventSemaphore's sequencer cost on the critical tail.
    nc.gpsimd.trigger_dma(count=1)._wait_ge(sOb, 1)

    # --- SP: the kernel-end gate.  SP's semaphore receive overhead is 0
    # and its sequencer decode is the cheapest, so the final wait on the
    # store-completion sem resolves earlier there than on Pool. ---
    nc.sync.wait_ge(sKV, 16)  # final store complete

    # --- DVE: init memsets, then the scan chain ---
    nc.vector.memset(dummy[:], 0.0).then_inc(sD, 1)
    nc.vector.memset(zb[:], 0.0).then_inc(sD, 1)
    nc.vector.memset(kvidx[:], 0).then_inc(sD, 1)
    nc.vector.wait_ge(sA, 2)
    # The DVE engine is freed before its SBUF write-ack returns, so every
    # same-engine RAW edge in this chain needs an explicit tick (sV) -- the
    # sem fires only after the ack, like Tile's engine-tick waits.
    # negu = (g-1)*c in one DVE op; the scan then computes
    # h = g*h - negu = g*h + (1-g)*c directly (op1=subtract).  At the reset
    # column T: g=0, c=0 => h[T]=0, so one scan covers both gate pairs.
    nc.vector.scalar_tensor_tensor(
        out=up[:], in0=gp[:], scalar=1.0, in1=cp[:],
        op0=OP.subtract, op1=OP.mult,
    ).then_inc(sV, 1)
    nc.vector.wait_ge(sV, 1)
    nc.vector.tensor_tensor_scan(
        out=hp[:], data0=gp[:], data1=up[:],
        initial=0.0, op0=OP.mult, op1=OP.subtract,
    ).then_inc(sV, 1)
    nc.vector.wait_ge(sV, 2)
    # wsum[h] = sum_t (h1+h2)*w == flat weighted sum over both halves
    nc.vector.tensor_mul(scrp[:], hp[:], wbp[:, 0:TC]).then_inc(sV, 1)
    nc.vector.wait_ge(sV, 3)
    with nc.allow_low_precision("33-term f32-accumulated reduce, fp16 store"):
        nc.vector.tensor_reduce(
            out=wsum16[:],
            in_=scrp[:],
            axis=mybir.AxisListType.X,
            op=OP.add,
        ).then_inc(sR, 1)
    nc.vector.wait_ge(sPo, 1)
    nc.vector.tensor_copy(ob[:], po[:]).then_inc(sOb, 1)

    # --- ACT: table-load hoist, then the two activations ---
    # dummy sigmoid right at the start makes bacc emit the (1.3us) ACT
    # table load during the input-DMA shadow.
    nc.scalar.wait_ge(sD, 1)   # dummy scratch ready
    nc.scalar.activation(out=dummy[:], in_=dummy[:], func=AF.Sigmoid)
    nc.scalar.wait_ge(sD, 2)   # zb (bias) ready
    nc.scalar.wait_ge(sG2, 1)  # gate matmuls drained
    nc.scalar.activation(
        out=gp[:], in_=pgg[:], func=AF.Sigmoid, bias=zb[:, 0:1]
    ).then_inc(sA, 1)
    nc.scalar.activation(
        out=cp[:], in_=pcc[:], func=AF.Tanh, bias=zb[:, 0:1]
    ).then_inc(sA, 1)

    # --- PE: warmup, gate matmuls, final GEMV ---
    # p-state warmup during the DMA shadow (on the zeroed zb scratch).
    nc.tensor.wait_ge(sD, 2)
    for _ in range(8):
        nc.tensor.matmul(pwarm[:], zb[0:1, 0:1], zb[0:1, 0:1], start=True, stop=True)
    nc.tensor.wait_ge(sP, 16)
    # paired gates: three matmuls per PSUM tile (pair1, reset col, pair2),
    # one activation over the whole row (biases ride in wgf row 100 against
    # the ones-row of xTw; the reset column's -30/0 rides row 101).
    nc.tensor.matmul(pgg[:, 0:T], wgf[:, 0 * HSL : 1 * HSL], xTw[:, 0:T],
                     start=True, stop=True)
    nc.tensor.matmul(pgg[:, T : T + 1], wgf[:, 0 * HSL : 1 * HSL],
                     xTw[:, T : T + 1], start=True, stop=True)
    nc.tensor.matmul(pgg[:, T + 1 : TC], wgf[:, 2 * HSL : 3 * HSL], xTw[:, 0:T],
                     start=True, stop=True)
    nc.tensor.matmul(pcc[:, 0:T], wgf[:, 1 * HSL : 2 * HSL], xTw[:, 0:T],
                     start=True, stop=True)
    nc.tensor.matmul(pcc[:, T : T + 1], wgf[:, 1 * HSL : 2 * HSL],
                     xTw[:, T : T + 1], start=True, stop=True)
    nc.tensor.matmul(pcc[:, T + 1 : TC], wgf[:, 3 * HSL : 4 * HSL], xTw[:, 0:T],
                     start=True, stop=True)
    nc.tensor.drain().then_inc(sG2, 1)
    nc.tensor.wait_ge(sR, 1)
    nc.tensor.wait_ge(sW, 16)
    w16v = wout[:, 0 : 2 * C16].bitcast(f16)
    w8v = wout[:, 2 * C16 : 2 * C16 + C8].bitcast(mybir.dt.float8e4)
    for j in range(NJ):
        # the last tile overlaps the previous one (cols OUT_D-128..OUT_D)
        # so every PSUM row is written with M=128; the host gather maps
        # column 20 to d = OUT_D-128+p.
        d0 = min(j * 128, OUT_D - 128)
        lhsT = w16v[:, d0 : d0 + 128] if j < NJ - NJ8 else w8v[:, d0 - C16 : d0 - C16 + 128]
        nc.tensor.matmul(
            po[:, j : j + 1],
            lhsT,
            wsum16[:],
            start=True,
            stop=True,
        )
    nc.tensor.drain().then_inc(sPo, 1)

    nc.finalize()
    return nc


def _build_nc_general():
    """Fallback: full pe stage on device (used when the rank-1 guard fails)."""
    import concourse.bacc as bacc
    import concourse.tile as tile
    from concourse import mybir

    f32 = mybir.dt.float32
    AF = mybir.ActivationFunctionType
    OP = mybir.AluOpType

    nc = bacc.Bacc("TRN2", target_bir_lowering=False, debug=False)

    d_p128 = nc.dram_tensor("pack128", [128, _P128_LEN], f32, kind="ExternalInput")
    d_pe = nc.dram_tensor("pe_pack", [128, 2 * T + 2 * NT], f32, kind="ExternalInput")
    d_w2t = nc.dram_tensor("w2t", [128, NT * IN_D], f32, kind="ExternalInput")
    d_p100 = nc.dram_tensor("pack100", [IN_D, _P100_LEN + 1], f32, kind="ExternalInput")
    d_wout = nc.dram_tensor("wout_t", [HSL, OUT_PAD], f32, kind="ExternalInput")
    d_out = nc.dram_tensor("out_part", [128, NJ], f32, kind="ExternalOutput")

    with tile.TileContext(nc) as tc:
        with (
            tc.tile_pool(name="cst", bufs=1) as cst,
            tc.tile_pool(name="pmm", bufs=2, space="PSUM") as pmm,
            tc.tile_pool(name="pout", bufs=1, space="PSUM") as pout,
        ):
            p128 = cst.tile([128, _P128_LEN], f32)
            nc.sync.dma_start(out=p128, in_=d_p128[:])
            pe_p = cst.tile([128, 2 * T + 2 * NT], f32)
            nc.sync.dma_start(out=pe_p, in_=d_pe[:])
            w2tt = cst.tile([128, NT * IN_D], f32)
            nc.sync.dma_start(out=w2tt, in_=d_w2t[:])
            p100 = cst.tile([IN_D, _P100_LEN + 1], f32)
            nc.sync.dma_start(out=p100, in_=d_p100[:])
            wout = cst.tile([HSL, OUT_PAD], f32)
            nc.scalar.dma_start(out=wout, in_=d_wout[:])

            wb = p128[:, _P128_WB:_P128_BGC]
            bgc = p128[:, _P128_BGC:_P128_LEN]
            posb = pe_p[:, 0:T]
            w1r = pe_p[:, 2 * T : 2 * T + NT]
            b1r = pe_p[:, 2 * T + NT : 2 * T + 2 * NT]
            w2t = w2tt.rearrange("p (n k) -> p n k", n=NT)

            tsT = p100[:, _P100_XT:_P100_WP1]
            b2c = p100[:, _P100_LEN : _P100_LEN + 1]
            wp1 = p100[:, _P100_WP1:_P100_BP1]
            bp1 = p100[:, _P100_BP1:_P100_WG]
            wgt = {}
            for gi, nm in enumerate(("g1", "c1", "g2", "c2")):
                o = _P100_WG + gi * NKC * HSL
                wgt[nm] = p100[:, o : o + NKC * HSL].rearrange(
                    "p (n m) -> p n m", n=NKC
                )

            # pe stage: peT[h, t] = relu(pos_t*w1[h]+b1[h]); pos_embT = sum_h
            peT = cst.tile([128, NT, T], f32)
            for i in range(NT):
                nc.scalar.activation(
                    out=peT[:, i, :],
                    in_=posb,
                    func=AF.Relu,
                    bias=b1r[:, i : i + 1],
                    scale=w1r[:, i : i + 1],
                )
            ppe = pmm.tile([IN_D, T], f32, tag="mm")
            for i in range(NT):
                nc.tensor.matmul(
                    ppe, w2t[:, i, :], peT[:, i, :], start=(i == 0), stop=(i == NT - 1)
                )
            xT = cst.tile([IN_D, T], f32)
            nc.vector.scalar_tensor_tensor(
                out=xT, in0=ppe, scalar=b2c[:, 0:1], in1=tsT, op0=OP.add, op1=OP.add
            )

            _build_core(
                nc, tile, mybir, tc, cst, pmm, pout,
                xT, wp1, bp1, wgt, bgc, wb, wout, d_out,
            )

    nc.finalize()
    return nc


def _prep_common(inputs):
    f = np.float32
    arr = {k: np.asarray(v, dtype=f) for k, v in inputs.items() if k != "positions"}
    pos = np.asarray(inputs["positions"]).astype(f)
    ts = arr["time_steps"]
    S = ts.shape[0]
    # softmax(arange(S,0,-1))[t] = exp(-t)/Z with Z the geometric sum.
    Z = (1.0 - np.exp(-float(S))) / (1.0 - np.exp(-1.0))
    w = (np.exp(-np.arange(T, dtype=np.float64)) / Z).astype(f)
    return arr, pos, w


def _core_p128(a, p128_base, sl):
    pc = p128_base.copy()
    pc[:, _P128_BGC + 0] = a["b_g1"][sl]
    pc[:, _P128_BGC + 1] = a["b_c1"][sl]
    pc[:, _P128_BGC + 2] = a["b_g2"][sl]
    pc[:, _P128_BGC + 3] = a["b_c2"][sl]
    return pc


def _core_wg(a, sl):
    wg = np.zeros((IN_D, _WG_LEN), np.float32)
    for gi, k in enumerate(("W_g1", "W_c1", "W_g2", "W_c2")):
        o = gi * NKC * HSL
        blk = a[k][sl].T.reshape(NKC, 100, HSL).transpose(1, 0, 2)
        wg[:, o : o + NKC * HSL] = blk.reshape(100, NKC * HSL)
    return wg


def _core_wout(a, sl, dtype=np.float16):
    wo = np.zeros((HSL, OUT_PAD), dtype)
    wo[:, :OUT_D] = a["W_out"][:, sl].T.astype(dtype)
    return wo


def _prep_inputs(inputs):
    """Host-side shard/layout prep. Returns (mode, per-core input maps, b_out)."""
    a, pos, w = _prep_common(inputs)
    ts = a["time_steps"]

    p128 = np.zeros((128, _P128_LEN), np.float32)
    p128[:, _P128_WB:_P128_BGC] = w[None, :]

    fast = bool((a["b_pe1"] == 0).all() and (pos[:T] >= 0).all())
    if fast:
        # rank-1 pos_emb folded into xT (see module docstring)
        v = a["W_pe2"] @ np.maximum(a["W_pe1"][:, 0], 0.0)
        xT = ts[:T].T + v[:, None] * pos[None, :T] + a["b_pe2"][:, None]
        # fold proj1 into the gate weights/biases (linear-layer composition)
        Wf = {k: a[k] @ a["W_p1"] for k in ("W_g1", "W_c1", "W_g2", "W_c2")}
        bf = {
            "b_g1": a["b_g1"] + a["W_g1"] @ a["b_p1"],
            "b_c1": a["b_c1"] + a["W_c1"] @ a["b_p1"],
            "b_g2": a["b_g2"] + a["W_g2"] @ a["b_p1"],
            "b_c2": a["b_c2"] + a["W_c2"] @ a["b_p1"],
        }
        in_maps = []
        for ci in range(NCORES):
            sl = slice(ci * HSL, (ci + 1) * HSL)
            pa1 = np.zeros((128, _P1_LEN), np.float32)
            h16 = pa1.view(np.float16)
            h16[:IN_D, 0 : T] = xT.astype(np.float16)
            h16[IN_D, 0 : T] = 1.0
            h16[IN_D + 1, T] = 1.0  # zero-gate reset column flag
            for gi, k in enumerate(("W_g1", "W_c1", "W_g2", "W_c2")):
                o = 2 * _P1_WGF + gi * HSL
                h16[:IN_D, o : o + HSL] = Wf[k][sl].T.astype(np.float16)
                h16[IN_D, o : o + HSL] = bf["b" + k[1:]][sl].astype(np.float16)
                # row 101 meets the reset column's flag: -30 drives the
                # gate sigmoid to 0; 0 keeps the candidate tanh at 0.
                h16[IN_D + 1, o : o + HSL] = -30.0 if k[2] == "g" else 0.0
            w16 = w.astype(np.float16)
            h16[:, 2 * _P1_WB : 2 * _P1_WB + T] = w16[None, :]
            h16[:, 2 * _P1_WB + T + 1 : 2 * _P1_WB + TC] = w16[None, :]
            import ml_dtypes
            wt = a["W_out"][:, sl].T  # (HSL, OUT_D)
            wo = np.zeros((HSL, WB_BYTES), np.uint8)
            wo[:, : 2 * C16] = np.ascontiguousarray(
                wt[:, :C16].astype(np.float16)).view(np.uint8)
            wo[:, 2 * C16 : 2 * C16 + C8] = np.ascontiguousarray(
                wt[:, C16:].astype(ml_dtypes.float8_e4m3)).view(np.uint8)
            in_maps.append({
                "pack1": pa1,
                "wout_t": wo,
            })
        return "fast", in_maps, a["b_out"]

    # general fallback: pe stage on device
    pe_p = np.zeros((128, 2 * T + 2 * NT), np.float32)
    pe_p[:, 0:T] = pos[None, :T]
    pe_p[:, 2 * T : 2 * T + NT] = a["W_pe1"][:, 0].reshape(NT, 128).T
    pe_p[:, 2 * T + NT : 2 * T + 2 * NT] = a["b_pe1"].reshape(NT, 128).T
    w2t = (
        a["W_pe2"].T.reshape(NT, 128, IN_D).transpose(1, 0, 2).reshape(128, NT * IN_D)
    ).copy()
    p100 = np.zeros((IN_D, _P100_LEN + 1), np.float32)
    p100[:, _P100_XT:_P100_WP1] = ts[:T].T
    p100[:, _P100_WP1:_P100_BP1] = a["W_p1"].T
    p100[:, _P100_BP1:_P100_WG] = a["b_p1"].reshape(NKC, 100).T
    p100[:, _P100_LEN] = a["b_pe2"]
    in_maps = []
    for ci in range(NCORES):
        sl = slice(ci * HSL, (ci + 1) * HSL)
        full = p100.copy()
        full[:, _P100_WG:_P100_LEN] = _core_wg(a, sl)
        in_maps.append({
            "pack128": _core_p128(a, p128, sl),
            "pack100": full,
            "pe_pack": pe_p,
            "w2t": w2t,
            "wout_t": _core_wout(a, sl, dtype=np.float32),
        })
    return "general", in_maps, a["b_out"]


def _run(inputs, trace=False):
    from concourse.bass_utils import run_bass_kernel_spmd

    mode, in_maps, b_out = _prep_inputs(inputs)
    key = f"nc_{mode}"
    if key not in _CACHE:
        _CACHE[key] = _build_nc_fast() if mode == "fast" else _build_nc_general()
    nc = _CACHE[key]
    res = run_bass_kernel_spmd(nc, in_maps, core_ids=list(range(NCORES)), trace=trace)
    acc = np.zeros(OUT_D, dtype=np.float32)
    for r in res.results:
        part = np.asarray(r["out_part"], dtype=np.float32).reshape(128, NJ)
        if mode == "fast":
            p = np.zeros(OUT_D, np.float32)
            p[: 128 * (NJ - 1)] = part[:, : NJ - 1].T.ravel()
            p[OUT_D - 128 :] = part[:, NJ - 1]
            acc = acc + p
        else:
            acc = acc + part.T.ravel()[:OUT_D]
    return (acc + b_out).astype(np.float32), res


def kernel(**inputs):
    out, _ = _run(inputs, trace=False)
    return out
